# revision 1
# baseline (speedup 1.0000x reference)
"""DirectedDualPNA on 8 Trainium2 NeuronCores.

Strategy (node-sharded):
  m_e = h_e @ pre_W + pre_b with h_e = [x_dst | x_src] decomposes as
  m_e = A[dst] + B[src],  A = x @ pre_W[:F] + pre_b,  B = x @ pre_W[F:].
  Per-dst segment stats of m then reduce to segment stats of B[src]:
    sum   = cnt*A + sum(B);   mean = sum/safe
    var   = E[B^2] - E[B]^2   (A shifts cancel)
    min   = A + min(B); max = A + max(B)   (0 when cnt==0)
  So only B rows (512B) are gathered per edge (dma_gather, int16 idx via a
  lo/hi table split), aggregated per 128-node groups (nodes degree-sorted so
  group slot-counts are tight), then the node-level PNA tail (post/lin via
  PE matmuls with the three degree scalers applied to matmul outputs).
  Each core owns N/8 dst nodes; B tables are computed replicated on every
  core; one AllGather moves layer-1 output between layers.
"""

import os
import numpy as np

import concourse.bass as bass
import concourse.mybir as mybir
from concourse import bacc
from concourse.bass_utils import run_bass_kernel_spmd
from concourse.tile import TileContext
from concourse.masks import make_identity

P = 128
F = 128
NCORES = 8
LAYERS = 2
EPS = 1e-5
BIG = 1e30
FP32 = mybir.dt.float32
I16 = mybir.dt.int16


# ----------------------------------------------------------------- host prep

def _wrap16(flat):
    """[n] int16 -> wrapped [128, n//16]: position j lives at (j%16, j//16),
    replicated across the 8 Q7 cores (every 16 partitions)."""
    n = flat.shape[0]
    assert n % 16 == 0
    w = flat.reshape(n // 16, 16).T.astype(np.int16)
    return np.tile(w, (8, 1))


def _prep_direction(edge_index, n_nodes, nown, half, avg_log):
    """Host-side per-direction prep. Returns per-core dict + shared K sched."""
    src = np.asarray(edge_index[0], dtype=np.int64)
    dst = np.asarray(edge_index[1], dtype=np.int64)
    nownp = ((nown + P - 1) // P) * P
    ng = nownp // P
    cores = []
    for c in range(NCORES):
        sel = (dst >= c * nown) & (dst < (c + 1) * nown)
        es = src[sel]
        ed = dst[sel] - c * nown
        cnt = np.bincount(ed, minlength=nown)
        order = np.argsort(ed, kind="stable")
        es_sorted = es[order]
        starts = np.zeros(nown + 1, np.int64)
        np.cumsum(cnt, out=starts[1:])
        perm = np.argsort(-cnt, kind="stable")
        lo_lists = []
        hi_lists = []
        for j in range(nownp):
            if j < nown:
                n = perm[j]
                s = es_sorted[starts[n]:starts[n + 1]]
                lo_lists.append(s[s < half])
                hi_lists.append(s[s >= half])
            else:
                lo_lists.append(np.empty(0, np.int64))
                hi_lists.append(np.empty(0, np.int64))
        clo = np.array([len(v) for v in lo_lists], np.int64)
        chi = np.array([len(v) for v in hi_lists], np.int64)
        cores.append(dict(cnt=cnt, perm=perm, lo=lo_lists, hi=hi_lists,
                          clo=clo, chi=chi))
    # shared slot schedule
    K_lo = np.zeros(ng, np.int64)
    K_hi = np.zeros(ng, np.int64)
    for g in range(ng):
        s, e = g * P, (g + 1) * P
        K_lo[g] = max(1, max(int(cc["clo"][s:e].max()) for cc in cores))
        K_hi[g] = max(1, max(int(cc["chi"][s:e].max()) for cc in cores))
    dummy_hi = n_nodes - half  # row `n_nodes` of the table, in hi-half coords
    out_cores = []
    for c in range(NCORES):
        cc = cores[c]
        idx_lo_parts = []
        idx_hi_parts = []
        scal = np.zeros((ng, P, 16), np.float32)
        for g in range(ng):
            kl, kh = int(K_lo[g]), int(K_hi[g])
            slo = np.zeros((kl, P), np.int64)
            shi = np.full((kh, P), dummy_hi, np.int64)
            for p in range(P):
                j = g * P + p
                lo, hi = cc["lo"][j], cc["hi"][j]
                nl_, nh_ = len(lo), len(hi)
                if nl_ > 0:
                    slo[:nl_, p] = lo
                    slo[nl_:, p] = lo[0]
                # else stays 0 (row 0; masked + sum-corrected)
                if nh_ > 0:
                    shi[:nh_, p] = hi - half
                    shi[nh_:, p] = hi[0] - half
                cntj = cc["cnt"][cc["perm"][j]] if j < nown else 0
                safe = max(cntj, 1)
                logd = np.log(safe + 1.0)
                scal[g, p, 0] = -(kl - nl_)          # npadlo
                scal[g, p, 1] = -(kh - nh_)          # npadhi
                scal[g, p, 2] = 0.0 if nl_ > 0 else BIG   # mlo_big
                scal[g, p, 3] = 0.0 if nh_ > 0 else BIG   # mhi_big
                scal[g, p, 4] = 1.0 / safe           # recip
                scal[g, p, 5] = 1.0 if cntj > 0 else 0.0  # nonempty
                scal[g, p, 6] = logd / avg_log       # f1
                scal[g, p, 7] = avg_log / logd       # f2
                scal[g, p, 8] = float(cntj)          # cnt
            idx_lo_parts.append(_wrap16(slo.reshape(-1).astype(np.int16)))
            idx_hi_parts.append(_wrap16(shi.reshape(-1).astype(np.int16)))
        perm_pad = np.concatenate([cc["perm"], np.zeros(nownp - nown, np.int64)])
        # AX gather: pos c*128+p -> row perm_pad[c*128+p] of natural A table
        ax_idx = _wrap16(perm_pad.astype(np.int16))
        # h scatter: pos c*128+p (perm position) -> natural row perm_pad[...]
        hdest = perm_pad.copy()
        hdest[nown:] = nownp  # dummy row
        h_idx = _wrap16(hdest.astype(np.int16))
        out_cores.append(dict(
            idx_lo=np.concatenate(idx_lo_parts, axis=1),
            idx_hi=np.concatenate(idx_hi_parts, axis=1),
            scal=scal, ax_idx=ax_idx, h_idx=h_idx,
        ))
    return out_cores, K_lo, K_hi, ng, nownp


def _prep_weights(inputs, l):
    """Per-layer/direction packed weights (numpy)."""
    w = {}
    for d, tag in enumerate(("in", "out")):
        preW = np.asarray(inputs[f"{tag}_pre_W"][l], np.float32)    # [256,128]
        preB = np.asarray(inputs[f"{tag}_pre_b"][l], np.float32)    # [128]
        postW = np.asarray(inputs[f"{tag}_post_W"][l], np.float32)  # [2048,64]
        postB = np.asarray(inputs[f"{tag}_post_b"][l], np.float32)  # [64]
        linW = np.asarray(inputs[f"{tag}_lin_W"][l], np.float32)    # [64,64]
        linB = np.asarray(inputs[f"{tag}_lin_b"][l], np.float32)    # [64]
        w[(d, "wbot")] = preW[F:2 * F]                               # [128,128]
        w[(d, "acatw")] = np.concatenate([preW[0:F], postW[0:F]], axis=1)  # [128,192]
        w[(d, "acatb")] = np.tile(np.concatenate([preB, postB])[None, :], (P, 1))
        pp = np.zeros((F, 5 * 192), np.float32)
        for p_ in range(5):
            for k in range(3):
                rows = postW[F + k * 5 * F + p_ * F: F + k * 5 * F + (p_ + 1) * F]
                pp[:, p_ * 192 + k * 64: p_ * 192 + (k + 1) * 64] = rows
        w[(d, "ppw")] = pp
        w[(d, "linw")] = linW
        w[(d, "linb")] = np.tile(linB[None, :], (P, 1))
    w["combw"] = np.asarray(inputs["comb_W"][l], np.float32)         # [256,128]
    w["combb"] = np.tile(np.asarray(inputs["comb_b"][l], np.float32)[None, :], (P, 1))
    return w


# -------------------------------------------------------------- device build

def _seg_fold(nc, scratch, src_tile, off, K, op, out_ap, vec):
    """out[128,F] = reduce(src_tile[:, off : off+K*F] viewed [K,F], op) along K.
    Pure-DVE contiguous fold tree (odd blocks deferred, no cross-engine
    copies)."""
    tt = vec.tensor_tensor
    if K == 1:
        vec.tensor_copy(out_ap, src_tile[:, off:off + F])
        return
    if K == 2:
        tt(out_ap, src_tile[:, off:off + F], src_tile[:, off + F:off + 2 * F], op=op)
        return
    if K == 3:
        tt(scratch[:, 0:F], src_tile[:, off:off + F],
           src_tile[:, off + F:off + 2 * F], op=op)
        tt(out_ap, scratch[:, 0:F], src_tile[:, off + 2 * F:off + 3 * F], op=op)
        return
    pend = []
    h = K // 2
    tt(scratch[:, 0:h * F], src_tile[:, off:off + h * F],
       src_tile[:, off + h * F:off + 2 * h * F], op=op)
    if K - 2 * h:
        pend.append((src_tile, off + 2 * h * F))
    cur = h
    while cur > 2:
        h2 = cur // 2
        tt(scratch[:, 0:h2 * F], scratch[:, 0:h2 * F],
           scratch[:, h2 * F:2 * h2 * F], op=op)
        if cur - 2 * h2:
            # odd block at 2*h2*F is never touched by deeper levels
            pend.append((scratch, 2 * h2 * F))
        cur = h2
    if cur == 2:
        operands = [(scratch, 0), (scratch, F)] + pend
    else:
        operands = [(scratch, 0)] + pend
    n = len(operands)
    at, ao = operands[0]
    for i in range(1, n):
        bt, bo = operands[i]
        dst = out_ap if i == n - 1 else scratch[:, 0:F]
        tt(dst, at[:, ao:ao + F], bt[:, bo:bo + F], op=op)
        at, ao = scratch, 0
    return


def build_program(meta):
    """Build the SPMD bass program (shared by all 8 cores)."""
    n_nodes = meta["n_nodes"]
    half = meta["half"]
    tbl = meta["tbl"]
    nown = meta["nown"]
    nownp = meta["nownp"]
    ng = meta["ng"]
    K_lo = meta["K_lo"]    # [L? no: per direction] dict d -> [ng]
    K_hi = meta["K_hi"]
    sum_klo = {d: int(K_lo[d].sum()) for d in (0, 1)}
    sum_khi = {d: int(K_hi[d].sum()) for d in (0, 1)}
    maxK = max(max(int(K_lo[d].max()), int(K_hi[d].max())) for d in (0, 1))
    ntile_x = tbl // P              # B-table build tiles
    ntile_x2 = (n_nodes + P - 1) // P   # x2 full tiles (rest zeroed)
    HT = nownp + P                  # h table rows (incl dummy)

    nc = bacc.Bacc("TRN2", target_bir_lowering=False, debug=False,
                   num_devices=NCORES)

    # ---- DRAM I/O
    xT = nc.dram_tensor("xT", [P, tbl], FP32, kind="ExternalInput")
    xTown = nc.dram_tensor("xTown", [P, nownp], FP32, kind="ExternalInput")
    idx_lo = {d: nc.dram_tensor(f"idx_lo{d}", [P, sum_klo[d] * 8], I16, kind="ExternalInput") for d in (0, 1)}
    idx_hi = {d: nc.dram_tensor(f"idx_hi{d}", [P, sum_khi[d] * 8], I16, kind="ExternalInput") for d in (0, 1)}
    scal_t = {d: nc.dram_tensor(f"scal{d}", [ng, P, 16], FP32, kind="ExternalInput") for d in (0, 1)}
    ax_idx_t = {d: nc.dram_tensor(f"axidx{d}", [P, nownp // 16], I16, kind="ExternalInput") for d in (0, 1)}
    h_idx_t = {d: nc.dram_tensor(f"hidx{d}", [P, nownp // 16], I16, kind="ExternalInput") for d in (0, 1)}
    wbot_t = nc.dram_tensor("wbot", [LAYERS, 2, F, F], FP32, kind="ExternalInput")
    acatw_t = nc.dram_tensor("acatw", [LAYERS, 2, F, 192], FP32, kind="ExternalInput")
    acatb_t = nc.dram_tensor("acatb", [LAYERS, 2, P, 192], FP32, kind="ExternalInput")
    ppw_t = nc.dram_tensor("ppw", [LAYERS, 2, F, 5 * 192], FP32, kind="ExternalInput")
    linw_t = nc.dram_tensor("linw", [LAYERS, 2, 64, 64], FP32, kind="ExternalInput")
    linb_t = nc.dram_tensor("linb", [LAYERS, 2, P, 64], FP32, kind="ExternalInput")
    combw_t = nc.dram_tensor("combw", [LAYERS, 256, F], FP32, kind="ExternalInput")
    combb_t = nc.dram_tensor("combb", [LAYERS, P, F], FP32, kind="ExternalInput")
    headw_t = nc.dram_tensor("headw", [F, 8], FP32, kind="ExternalInput")
    headb_t = nc.dram_tensor("headb", [P, 8], FP32, kind="ExternalInput")
    out_t = nc.dram_tensor("out", [nown, 8], FP32, kind="ExternalOutput")

    # ---- DRAM internal
    Bt = {d: nc.dram_tensor(f"Bt{d}", [tbl, F], FP32) for d in (0, 1)}
    Atab = nc.dram_tensor("Atab", [nownp, 192], FP32)
    h_tab = {d: nc.dram_tensor(f"htab{d}", [HT, 64], FP32) for d in (0, 1)}
    x2own_int = nc.dram_tensor("x2own", [nown, F], FP32)
    x2full = nc.dram_tensor("x2full", [n_nodes, F], FP32, addr_space="Shared")

    AF = mybir.ActivationFunctionType
    OP = mybir.AluOpType
    AX_ = mybir.AxisListType

    with TileContext(nc) as tc:
        with tc.tile_pool(name="const", bufs=1) as constp, \
             tc.tile_pool(name="pers", bufs=1) as persp, \
             tc.tile_pool(name="wts", bufs=1) as wtsp, \
             tc.tile_pool(name="xt", bufs=2) as xtp, \
             tc.tile_pool(name="gath", bufs=2) as gathp, \
             tc.tile_pool(name="sqp", bufs=1) as sqp, \
             tc.tile_pool(name="fold", bufs=2) as foldp, \
             tc.tile_pool(name="ip", bufs=3) as ipool, \
             tc.tile_pool(name="nl", bufs=2) as nlp, \
             tc.tile_pool(name="ps", bufs=2, space="PSUM") as psp, \
             tc.tile_pool(name="psa", bufs=2, space="PSUM") as psap:

            ident = constp.tile([P, P], FP32)
            make_identity(nc, ident[:])
            zeros = constp.tile([P, 1024], FP32)
            nc.vector.memset(zeros[:], 0.0)
            eps_col = constp.tile([P, 1], FP32)
            nc.vector.memset(eps_col[:], EPS)

            AXt = persp.tile([P, ng * 192], FP32)         # gathered A|X0 (perm)
            hsb = persp.tile([P, ng * 64], FP32)          # h accumulation (perm)

            def load_w(pool, dram_ap, shape, tag):
                t = pool.tile(shape, FP32, tag=tag)
                nc.sync.dma_start(out=t[:], in_=dram_ap)
                return t

            headw_sb = load_w(constp, headw_t[:], [F, 8], "headw")
            headb_sb = load_w(constp, headb_t[:], [P, 8], "headb")

            def zero_dram(tensor, rows, width):
                flat = tensor[:].rearrange("n f -> (n f)")
                total = rows * width
                assert total % P == 0
                per = total // P
                v = flat.rearrange("(p x) -> p x", p=P)
                off = 0
                while off < per:
                    w = min(1024, per - off)
                    nc.sync.dma_start(out=v[:, off:off + w], in_=zeros[:, 0:w])
                    off += w

            def transpose_to_sbuf(src_ap, rows, cols, tag):
                """PE transpose src [rows, cols] -> sbuf [cols, rows]."""
                pt = psp.tile([P, P], FP32, tag="ptr")
                nc.tensor.transpose(out=pt[:cols, :rows], in_=src_ap, identity=ident[:])
                sb = xtp.tile([P, P], FP32, tag=tag)
                nc.vector.tensor_copy(sb[:cols, :rows], pt[:cols, :rows])
                return sb

            for l in range(LAYERS):
                wbot_sb = {d: load_w(wtsp, wbot_t[l, d], [F, F], f"wbot{d}") for d in (0, 1)}
                combw1_sb = load_w(wtsp, combw_t[l, 0:F, :], [F, F], "combw1")
                combw2_sb = load_w(wtsp, combw_t[l, F:256, :], [F, F], "combw2")
                combb_sb = load_w(wtsp, combb_t[l], [P, F], "combb")

                # ---------- phase A: B tables (both directions)
                for t in range(ntile_x if l == 0 else ntile_x2):
                    if l == 0:
                        lhsT = xtp.tile([P, P], FP32, tag="xtt")
                        nc.sync.dma_start(out=lhsT[:], in_=xT[:, t * P:(t + 1) * P])
                    else:
                        rows = min(P, n_nodes - t * P)
                        x2t = xtp.tile([P, P], FP32, tag="x2t")
                        if rows < P:
                            nc.vector.memset(x2t[:], 0.0)
                        nc.sync.dma_start(out=x2t[:rows, :],
                                          in_=x2full[t * P:t * P + rows, :])
                        lhsT = transpose_to_sbuf(x2t[:], P, P, "xtt")
                    for d in (0, 1):
                        pb = psp.tile([P, P], FP32, tag="pb")
                        nc.tensor.matmul(out=pb[:], lhsT=lhsT[:], rhs=wbot_sb[d][:],
                                         start=True, stop=True)
                        sb = xtp.tile([P, P], FP32, tag="bsb")
                        nc.scalar.copy(sb[:], pb[:])
                        rows_o = min(P, tbl - t * P)
                        nc.sync.dma_start(out=Bt[d][t * P:t * P + rows_o, :],
                                          in_=sb[:rows_o, :])
                if l == 1:
                    # zero rows n_nodes..tbl of both tables (dummy row etc.)
                    for d in (0, 1):
                        r = n_nodes
                        while r < tbl:
                            w = min(P, tbl - r)
                            nc.sync.dma_start(
                                out=Bt[d][r:r + w, :],
                                in_=zeros[:w, 0:F])
                            r += w

                for d in (0, 1):
                    # ---------- phase B: A|X0 natural table + perm gather
                    acatw_sb = load_w(wtsp, acatw_t[l, d], [F, 192], "acatw")
                    acatb_sb = load_w(wtsp, acatb_t[l, d], [P, 192], "acatb")
                    ppw_sb = load_w(wtsp, ppw_t[l, d], [F, 5 * 192], "ppw")
                    linw_sb = load_w(wtsp, linw_t[l, d], [64, 64], "linw")
                    linb_sb = load_w(wtsp, linb_t[l, d], [P, 64], "linb")

                    for g in range(ng):
                        if l == 0:
                            lhsT = xtp.tile([P, P], FP32, tag="xtt")
                            nc.sync.dma_start(out=lhsT[:],
                                              in_=xTown[:, g * P:(g + 1) * P])
                        else:
                            rows_b = min(P, nown - g * P)
                            x2t = xtp.tile([P, P], FP32, tag="x2t")
                            if rows_b < P:
                                nc.vector.memset(x2t[:], 0.0)
                            nc.sync.dma_start(out=x2t[:rows_b, :],
                                              in_=x2own_int[g * P:g * P + rows_b, :])
                            lhsT = transpose_to_sbuf(x2t[:], P, P, "xtt")
                        pa = psap.tile([P, 192], FP32, tag="pa")
                        nc.tensor.matmul(out=pa[:], lhsT=lhsT[:], rhs=acatw_sb[:],
                                         start=True, stop=True)
                        arow = xtp.tile([P, 192], FP32, tag="arow")
                        nc.vector.tensor_add(arow[:], pa[:], acatb_sb[:])
                        nc.sync.dma_start(out=Atab[g * P:(g + 1) * P, :], in_=arow[:])
                    axi = ipool.tile([P, nownp // 16], I16, tag="axi")
                    nc.sync.dma_start(out=axi[:], in_=ax_idx_t[d][:])
                    nc.gpsimd.dma_gather(
                        out_ap=AXt[:].rearrange("p (c w) -> p c w", w=192),
                        in_ap=Atab[:], idxs_ap=axi[:],
                        num_idxs=nownp, num_idxs_reg=nownp,
                        elem_size=192, single_packet=False)

                    # ---------- phase C: gather + stats + post/lin per group
                    off_lo = 0
                    off_hi = 0
                    kl_arr, kh_arr = K_lo[d], K_hi[d]
                    for g in range(ng):
                        KL, KH = int(kl_arr[g]), int(kh_arr[g])
                        Wd = (KL + KH) * F
                        il = ipool.tile([P, KL * 8], I16, tag="il")
                        nc.sync.dma_start(out=il[:], in_=idx_lo[d][:, off_lo:off_lo + KL * 8])
                        ih = ipool.tile([P, KH * 8], I16, tag="ih")
                        nc.sync.dma_start(out=ih[:], in_=idx_hi[d][:, off_hi:off_hi + KH * 8])
                        off_lo += KL * 8
                        off_hi += KH * 8
                        sc = ipool.tile([P, 16], FP32, tag="sc")
                        nc.sync.dma_start(out=sc[:], in_=scal_t[d][g])
                        gt = gathp.tile([P, (maxK * 2) * F], FP32, tag="gt")
                        nc.gpsimd.dma_gather(
                            out_ap=gt[:, 0:KL * F].rearrange("p (k f) -> p k f", f=F),
                            in_ap=Bt[d][0:half, :], idxs_ap=il[:],
                            num_idxs=KL * P, num_idxs_reg=KL * P,
                            elem_size=F, single_packet=False)
                        nc.gpsimd.dma_gather(
                            out_ap=gt[:, KL * F:Wd].rearrange("p (k f) -> p k f", f=F),
                            in_ap=Bt[d][half:tbl, :], idxs_ap=ih[:],
                            num_idxs=KH * P, num_idxs_reg=KH * P,
                            elem_size=F, single_packet=False)
                        fsc = foldp.tile([P, (maxK // 2 + 2) * F], FP32, tag="fsc")

                        def nlt(tag):
                            return nlp.tile([P, F], FP32, tag=tag, name=tag)

                        slo, shi = nlt("slo"), nlt("shi")
                        qlo, qhi = nlt("qlo"), nlt("qhi")
                        mnlo, mnhi = nlt("mnlo"), nlt("mnhi")
                        mxlo, mxhi = nlt("mxlo"), nlt("mxhi")
                        v = nc.vector
                        _seg_fold(nc, fsc, gt, 0, KL, OP.add, slo[:], v)
                        _seg_fold(nc, fsc, gt, KL * F, KH, OP.add, shi[:], v)
                        sq = sqp.tile([P, maxK * F], FP32, tag="sq")
                        nc.scalar.activation(sq[:, 0:KL * F], gt[:, 0:KL * F], AF.Square)
                        _seg_fold(nc, fsc, sq, 0, KL, OP.add, qlo[:], v)
                        sq2 = sqp.tile([P, maxK * F], FP32, tag="sq")
                        nc.scalar.activation(sq2[:, 0:KH * F], gt[:, KL * F:Wd], AF.Square)
                        _seg_fold(nc, fsc, sq2, 0, KH, OP.add, qhi[:], v)
                        _seg_fold(nc, fsc, gt, 0, KL, OP.min, mnlo[:], v)
                        _seg_fold(nc, fsc, gt, KL * F, KH, OP.min, mnhi[:], v)
                        _seg_fold(nc, fsc, gt, 0, KL, OP.max, mxlo[:], v)
                        _seg_fold(nc, fsc, gt, KL * F, KH, OP.max, mxhi[:], v)

                        s0lo = gt[:, 0:F]
                        s0hi = gt[:, KL * F:KL * F + F]
                        q0lo, q0hi = nlt("q0lo"), nlt("q0hi")
                        v.tensor_mul(q0lo[:], s0lo, s0lo)
                        v.tensor_mul(q0hi[:], s0hi, s0hi)
                        npl = sc[:, 0:1]
                        nph = sc[:, 1:2]
                        mlb = sc[:, 2:3]
                        mhb = sc[:, 3:4]
                        rcp = sc[:, 4:5]
                        nemp = sc[:, 5:6]
                        f1 = sc[:, 6:7]
                        f2 = sc[:, 7:8]
                        cntc = sc[:, 8:9]

                        t1, t2 = nlt("t1"), nlt("t2")
                        Sb, SQb = nlt("Sb"), nlt("SQb")
                        v.scalar_tensor_tensor(t1[:], s0lo, npl, slo[:], op0=OP.mult, op1=OP.add)
                        v.scalar_tensor_tensor(t2[:], s0hi, nph, shi[:], op0=OP.mult, op1=OP.add)
                        v.tensor_add(Sb[:], t1[:], t2[:])
                        v.scalar_tensor_tensor(t1[:], q0lo[:], npl, qlo[:], op0=OP.mult, op1=OP.add)
                        v.scalar_tensor_tensor(t2[:], q0hi[:], nph, qhi[:], op0=OP.mult, op1=OP.add)
                        v.tensor_add(SQb[:], t1[:], t2[:])
                        MN, MX = nlt("MN"), nlt("MX")
                        v.tensor_scalar(t1[:], mnlo[:], mlb, None, op0=OP.add)
                        v.tensor_scalar(t2[:], mnhi[:], mhb, None, op0=OP.add)
                        v.tensor_tensor(MN[:], t1[:], t2[:], op=OP.min)
                        v.tensor_scalar(t1[:], mxlo[:], mlb, None, op0=OP.subtract)
                        v.tensor_scalar(t2[:], mxhi[:], mhb, None, op0=OP.subtract)
                        v.tensor_tensor(MX[:], t1[:], t2[:], op=OP.max)

                        Ag = AXt[:, g * 192:g * 192 + F]
                        X0g = AXt[:, g * 192 + F:g * 192 + 192]
                        s_full, mean = nlt("s_full"), nlt("mean")
                        meanB, std = nlt("meanB"), nlt("std")
                        mn, mx = nlt("mn"), nlt("mx")
                        v.scalar_tensor_tensor(s_full[:], Ag, cntc, Sb[:], op0=OP.mult, op1=OP.add)
                        nc.scalar.activation(mean[:], s_full[:], AF.Copy, scale=rcp)
                        nc.scalar.activation(meanB[:], Sb[:], AF.Copy, scale=rcp)
                        nc.scalar.activation(t1[:], SQb[:], AF.Copy, scale=rcp)
                        v.tensor_mul(t2[:], meanB[:], meanB[:])
                        vr1, vr2 = nlt("vr1"), nlt("vr2")
                        v.tensor_sub(vr1[:], t1[:], t2[:])
                        v.tensor_scalar_max(vr2[:], vr1[:], 0.0)
                        nc.scalar.activation(std[:], vr2[:], AF.Sqrt, bias=eps_col[:, 0:1])
                        v.tensor_add(t1[:], Ag, MN[:])
                        v.tensor_scalar(mn[:], t1[:], nemp, None, op0=OP.mult)
                        v.tensor_add(t2[:], Ag, MX[:])
                        v.tensor_scalar(mx[:], t2[:], nemp, None, op0=OP.mult)

                        # post: y = X0 + sum_p sum_k f_k*(part_p @ Wp_k)
                        py = psap.tile([P, 192], FP32, tag="pa")
                        for pi, part in enumerate((mean, s_full, std, mn, mx)):
                            pt = psp.tile([P, P], FP32, tag="ptr")
                            nc.tensor.transpose(out=pt[:], in_=part[:], identity=ident[:])
                            partT = xtp.tile([P, P], FP32, tag="partT")
                            v.tensor_copy(partT[:], pt[:])
                            nc.tensor.matmul(out=py[:], lhsT=partT[:],
                                             rhs=ppw_sb[:, pi * 192:(pi + 1) * 192],
                                             start=(pi == 0), stop=(pi == 4))
                        yt, y64 = nlt("yt"), nlt("y64")
                        pys = nlp.tile([P, 192], FP32, tag="pys", name="pys")
                        nc.scalar.copy(pys[:], py[:])
                        v.scalar_tensor_tensor(yt[:, 0:64], pys[:, 64:128], f1,
                                               pys[:, 0:64], op0=OP.mult, op1=OP.add)
                        v.scalar_tensor_tensor(yt[:, 64:128], pys[:, 128:192], f2,
                                               X0g, op0=OP.mult, op1=OP.add)
                        v.tensor_add(y64[:, 0:64], yt[:, 0:64], yt[:, 64:128])
                        # lin + relu
                        pt = psp.tile([P, P], FP32, tag="ptr")
                        nc.tensor.transpose(out=pt[:64, :], in_=y64[:, 0:64], identity=ident[:])
                        ylhs = xtp.tile([64, P], FP32, tag="ylhs")
                        v.tensor_copy(ylhs[:], pt[:64, :])
                        pz = psp.tile([P, 64], FP32, tag="pz")
                        nc.tensor.matmul(out=pz[:], lhsT=ylhs[:], rhs=linw_sb[:],
                                         start=True, stop=True)
                        zb = nlt("zb")
                        v.tensor_add(zb[:, 0:64], pz[:], linb_sb[:, 0:64])
                        nc.scalar.activation(hsb[:, g * 64:(g + 1) * 64],
                                             zb[:, 0:64], AF.Relu)

                    # scatter h (perm -> natural)
                    zero_dram(h_tab[d], HT, 64)
                    hix = ipool.tile([P, nownp // 16], I16, tag="hix")
                    nc.sync.dma_start(out=hix[:], in_=h_idx_t[d][:])
                    nc.gpsimd.dma_scatter_add(
                        out_ap=h_tab[d][:],
                        in_ap=hsb[:].rearrange("p (c w) -> p c w", w=64),
                        idxs_ap=hix[:], num_idxs=nownp, num_idxs_reg=nownp,
                        elem_size=64, single_packet=False)

                # ---------- phase D: comb (+ head when l==1)
                for g in range(ng):
                    rows = min(P, nown - g * P)
                    if l == 0:
                        xlhs = xtp.tile([P, P], FP32, tag="xtt")
                        nc.sync.dma_start(out=xlhs[:],
                                          in_=xTown[:, g * P:(g + 1) * P])
                    else:
                        x2t = xtp.tile([P, P], FP32, tag="x2t")
                        if rows < P:
                            nc.vector.memset(x2t[:], 0.0)
                        nc.sync.dma_start(out=x2t[:rows, :],
                                          in_=x2own_int[g * P:g * P + rows, :])
                        xlhs = transpose_to_sbuf(x2t[:], P, P, "xtt")
                    hcatT = xtp.tile([P, P], FP32, tag="hcatT")
                    for d in (0, 1):
                        htile = xtp.tile([P, 64], FP32, tag="htile")
                        nc.sync.dma_start(out=htile[:], in_=h_tab[d][g * P:(g + 1) * P, :])
                        pt = psp.tile([P, P], FP32, tag="ptr")
                        nc.tensor.transpose(out=pt[:64, :], in_=htile[:], identity=ident[:])
                        nc.vector.tensor_copy(hcatT[d * 64:(d + 1) * 64, :], pt[:64, :])
                    pc = psp.tile([P, P], FP32, tag="pb")
                    nc.tensor.matmul(out=pc[:], lhsT=xlhs[:], rhs=combw1_sb[:],
                                     start=True, stop=False)
                    nc.tensor.matmul(out=pc[:], lhsT=hcatT[:], rhs=combw2_sb[:],
                                     start=False, stop=True)
                    xn = nlp.tile([P, F], FP32, tag="xn")
                    nc.vector.tensor_add(xn[:], pc[:], combb_sb[:])
                    if l == 0:
                        x2n = nlp.tile([P, F], FP32, tag="x2n", name="x2n")
                        nc.scalar.activation(x2n[:], xn[:], AF.Relu)
                        nc.sync.dma_start(out=x2own_int[g * P:g * P + rows, :],
                                          in_=x2n[:rows, :])
                    else:
                        x3 = nlp.tile([P, F], FP32, tag="x3")
                        nc.scalar.activation(x3[:], xn[:], AF.Relu)
                        pt = psp.tile([P, P], FP32, tag="ptr")
                        nc.tensor.transpose(out=pt[:], in_=x3[:], identity=ident[:])
                        x3T = xtp.tile([P, P], FP32, tag="x3T")
                        nc.vector.tensor_copy(x3T[:], pt[:])
                        ph = psp.tile([P, 8], FP32, tag="pz")
                        nc.tensor.matmul(out=ph[:], lhsT=x3T[:], rhs=headw_sb[:],
                                         start=True, stop=True)
                        ot = nlp.tile([P, 8], FP32, tag="ot")
                        nc.vector.tensor_add(ot[:], ph[:], headb_sb[:])
                        nc.sync.dma_start(out=out_t[g * P:g * P + rows, :],
                                          in_=ot[:rows, :])

                if l == 0:
                    nc.gpsimd.collective_compute(
                        "AllGather", OP.bypass,
                        replica_groups=[list(range(NCORES))],
                        ins=[x2own_int[:]], outs=[x2full[:]])

    nc.finalize()
    return nc


# ----------------------------------------------------------------- kernel()

def _install_ntff_hook():
    """Register the axon NTFF profile hook if the image's antenv lacks it."""
    import sys
    import types
    try:
        from antenv.axon_hooks import get_axon_ntff_profile_hook  # noqa: F401
        return
    except ImportError:
        pass
    try:
        mod = types.ModuleType("antenv.axon_hooks")
        hook = {"h": None}
        mod.set_axon_ntff_profile_hook = lambda h: hook.__setitem__("h", h)
        mod.get_axon_ntff_profile_hook = lambda: hook["h"]
        sys.modules["antenv.axon_hooks"] = mod
        import antenv
        antenv.axon_hooks = mod
        from trn_agent_boot.trn_boot import _ntff_profile_via_ctypes
        mod.set_axon_ntff_profile_hook(
            _ntff_profile_via_ctypes("/opt/axon/libaxon_pjrt.so"))
    except Exception:
        pass


def kernel(**inputs):
    x = np.asarray(inputs["x"], np.float32)
    n_nodes, f = x.shape
    assert f == F
    assert n_nodes % NCORES == 0
    nown = n_nodes // NCORES
    half = ((n_nodes // 2 + 1 + P - 1) // P) * P
    assert half < 32768 and 2 * half > n_nodes
    tbl = 2 * half

    avg_in = float(np.asarray(inputs["avg_in"]))
    avg_out = float(np.asarray(inputs["avg_out"]))

    prep = {}
    Ksched = {}
    for d, (ei, avg) in enumerate(
            ((inputs["edge_index_in"], avg_in), (inputs["edge_index_out"], avg_out))):
        cores, K_lo, K_hi, ng, nownp = _prep_direction(ei, n_nodes, nown, half, avg)
        prep[d] = cores
        Ksched[d] = (K_lo, K_hi)

    meta = dict(n_nodes=n_nodes, half=half, tbl=tbl, nown=nown, nownp=nownp,
                ng=ng, K_lo={d: Ksched[d][0] for d in (0, 1)},
                K_hi={d: Ksched[d][1] for d in (0, 1)})
    nc = build_program(meta)

    xT_np = np.zeros((P, tbl), np.float32)
    xT_np[:, :n_nodes] = x.T
    wl = [_prep_weights(inputs, l) for l in range(LAYERS)]
    wbot_np = np.stack([np.stack([wl[l][(d, "wbot")] for d in (0, 1)]) for l in range(LAYERS)])
    acatw_np = np.stack([np.stack([wl[l][(d, "acatw")] for d in (0, 1)]) for l in range(LAYERS)])
    acatb_np = np.stack([np.stack([wl[l][(d, "acatb")] for d in (0, 1)]) for l in range(LAYERS)])
    ppw_np = np.stack([np.stack([wl[l][(d, "ppw")] for d in (0, 1)]) for l in range(LAYERS)])
    linw_np = np.stack([np.stack([wl[l][(d, "linw")] for d in (0, 1)]) for l in range(LAYERS)])
    linb_np = np.stack([np.stack([wl[l][(d, "linb")] for d in (0, 1)]) for l in range(LAYERS)])
    combw_np = np.stack([wl[l]["combw"] for l in range(LAYERS)])
    combb_np = np.stack([wl[l]["combb"] for l in range(LAYERS)])
    headw_np = np.asarray(inputs["head_W"], np.float32)
    headb_np = np.tile(np.asarray(inputs["head_b"], np.float32)[None, :], (P, 1))

    in_maps = []
    for c in range(NCORES):
        xTown_np = np.zeros((P, meta["nownp"]), np.float32)
        xTown_np[:, :nown] = x[c * nown:(c + 1) * nown].T
        m = dict(xT=xT_np, xTown=xTown_np, wbot=wbot_np, acatw=acatw_np, acatb=acatb_np,
                 ppw=ppw_np, linw=linw_np, linb=linb_np, combw=combw_np,
                 combb=combb_np, headw=headw_np, headb=headb_np)
        for d in (0, 1):
            pc = prep[d][c]
            m[f"idx_lo{d}"] = pc["idx_lo"]
            m[f"idx_hi{d}"] = pc["idx_hi"]
            m[f"scal{d}"] = pc["scal"]
            m[f"axidx{d}"] = pc["ax_idx"]
            m[f"hidx{d}"] = pc["h_idx"]
        in_maps.append(m)

    trace = bool(int(os.environ.get("PNA_TRACE", "0")))
    if trace:
        _install_ntff_hook()
    res = run_bass_kernel_spmd(nc, in_maps, core_ids=list(range(NCORES)),
                               trace=trace)
    if trace and res.exec_time_ns is not None:
        print(f"HW exec time: {res.exec_time_ns} ns")
    out = np.concatenate([res.results[c]["out"] for c in range(NCORES)], axis=0)
    return out.astype(np.float32)


# Optional: expose sim path for debugging (used by test.py on small inputs)
def kernel_sim(**inputs):
    """Single-core-per-core simulation via MultiCoreSim (slow; small inputs)."""
    from concourse.bass_interp import MultiCoreSim
    x = np.asarray(inputs["x"], np.float32)
    n_nodes = x.shape[0]
    nown = n_nodes // NCORES
    half = ((n_nodes // 2 + 1 + P - 1) // P) * P
    tbl = 2 * half
    avg_in = float(np.asarray(inputs["avg_in"]))
    avg_out = float(np.asarray(inputs["avg_out"]))
    prep = {}
    Ksched = {}
    for d, (ei, avg) in enumerate(
            ((inputs["edge_index_in"], avg_in), (inputs["edge_index_out"], avg_out))):
        cores, K_lo, K_hi, ng, nownp = _prep_direction(ei, n_nodes, nown, half, avg)
        prep[d] = cores
        Ksched[d] = (K_lo, K_hi)
    meta = dict(n_nodes=n_nodes, half=half, tbl=tbl, nown=nown, nownp=nownp,
                ng=ng, K_lo={d: Ksched[d][0] for d in (0, 1)},
                K_hi={d: Ksched[d][1] for d in (0, 1)})
    nc = build_program(meta)

    xT_np = np.zeros((P, tbl), np.float32)
    xT_np[:, :n_nodes] = x.T
    wl = [_prep_weights(inputs, l) for l in range(LAYERS)]
    wbot_np = np.stack([np.stack([wl[l][(d, "wbot")] for d in (0, 1)]) for l in range(LAYERS)])
    acatw_np = np.stack([np.stack([wl[l][(d, "acatw")] for d in (0, 1)]) for l in range(LAYERS)])
    acatb_np = np.stack([np.stack([wl[l][(d, "acatb")] for d in (0, 1)]) for l in range(LAYERS)])
    ppw_np = np.stack([np.stack([wl[l][(d, "ppw")] for d in (0, 1)]) for l in range(LAYERS)])
    linw_np = np.stack([np.stack([wl[l][(d, "linw")] for d in (0, 1)]) for l in range(LAYERS)])
    linb_np = np.stack([np.stack([wl[l][(d, "linb")] for d in (0, 1)]) for l in range(LAYERS)])
    combw_np = np.stack([wl[l]["combw"] for l in range(LAYERS)])
    combb_np = np.stack([wl[l]["combb"] for l in range(LAYERS)])
    headw_np = np.asarray(inputs["head_W"], np.float32)
    headb_np = np.tile(np.asarray(inputs["head_b"], np.float32)[None, :], (P, 1))

    sim = MultiCoreSim(nc, num_cores=NCORES, trace=False,
                       require_finite=False, require_nnan=False)
    for c in range(NCORES):
        cs = sim.cores[c]
        cs.tensor("xT")[:] = xT_np
        xTown_np = np.zeros((P, nownp), np.float32)
        xTown_np[:, :nown] = x[c * nown:(c + 1) * nown].T
        cs.tensor("xTown")[:] = xTown_np
        for nm, val in (("wbot", wbot_np), ("acatw", acatw_np), ("acatb", acatb_np),
                        ("ppw", ppw_np), ("linw", linw_np), ("linb", linb_np),
                        ("combw", combw_np), ("combb", combb_np),
                        ("headw", headw_np), ("headb", headb_np)):
            cs.tensor(nm)[:] = val
        for d in (0, 1):
            pc = prep[d][c]
            cs.tensor(f"idx_lo{d}")[:] = pc["idx_lo"]
            cs.tensor(f"idx_hi{d}")[:] = pc["idx_hi"]
            cs.tensor(f"scal{d}")[:] = pc["scal"]
            cs.tensor(f"axidx{d}")[:] = pc["ax_idx"]
            cs.tensor(f"hidx{d}")[:] = pc["h_idx"]
    sim.simulate(check_with_hw=False)
    out = np.concatenate([np.array(sim.cores[c].tensor("out")) for c in range(NCORES)], axis=0)
    return out.astype(np.float32)



# revision 16
# speedup vs baseline: 2.0941x; 2.0941x over previous
"""DirectedDualPNA on 8 Trainium2 NeuronCores — v2.

Strategy (node-sharded, fused group pipeline):
  m_e = A[dst] + B[src] decomposition (pre-MLP splits over the concat), so per
  edge only the 512B B row is gathered.  All per-node tables live in one fused
  [50000, 256] DRAM table T = [B_in | B_out] in a custom row order:

  - nodes are globally degree-sorted (key = max(deg_in, deg_out)); rank r maps
    to core r%8.  Each core's groups of 128 nodes are degree-homogeneous, so
    the shared SPMD slot schedule K[g] is tight across cores.
  - a gather's int16 index addresses at most 32768 rows, so each group issues
    two gathers: window LO = rows [0, 32768), window HI = rows [N-32768, N).
    Which (core,pos) slot a source node occupies decides its window zone; ties
    in the degree sort are broken to (a) put high-traffic sources in the
    overlap zone (edges become window-flexible) and (b) 2-color the rest to
    balance every destination list's forced lo/hi counts (greedy discrepancy).
  - per group, per direction: gather lo+hi slots, one Square pass (scalar
    engine), then strided tensor_reduce folds (sum/sumsq full-span, min/max
    full-span or per-side when a group contains empty sides), then the PNA
    tail (post/lin) via PE matmuls, comb fused in-loop (no h scatter), and the
    next layer's B rows produced immediately (x2 @ Wbot) into T1own.
  - one AllGather of T1own rows between layers; layer-0 tables (T0, A0) are
    precomputed on the host.  Output rows are un-permuted on the host.
"""

import os
import numpy as np

import concourse.bass as bass
import concourse.mybir as mybir
from concourse import bacc
from concourse.bass_utils import run_bass_kernel_spmd
from concourse.tile import TileContext
from concourse.masks import make_identity

P = 128
F = 128
NCORES = 8
LAYERS = 2
EPS = 1e-5
BIG = 1e30
WIN = 32768
FP32 = mybir.dt.float32
I16 = mybir.dt.int16


# ----------------------------------------------------------------- host prep

def _wrap16(flat):
    """[n] int16 -> wrapped [128, n//16]: position j lives at (j%16, j//16),
    replicated across the 8 Q7 cores (every 16 partitions)."""
    n = flat.shape[0]
    assert n % 16 == 0
    w = flat.reshape(n // 16, 16).T.astype(np.int16)
    return np.tile(w, (8, 1))


def _assign_nodes(eis, n_nodes, nown, hi_start, lo_end):
    """Global degree sort + zone-aware slot assignment.

    Returns core_asg, pos_asg (per natural node) and row (table row)."""
    N = n_nodes
    cin = np.bincount(eis[0][1], minlength=N)
    cout = np.bincount(eis[1][1], minlength=N)
    key = np.maximum(cin, cout)
    rank_of = np.argsort(-key, kind="stable")          # rank i -> node
    refs = np.bincount(eis[0][0], minlength=N) + np.bincount(eis[1][0], minlength=N)

    # per-src edge CSR over (dir, dst) for the greedy coloring
    src_all = np.concatenate([eis[0][0], eis[1][0]])
    enc_all = np.concatenate([eis[0][1], N + eis[1][1]])
    order = np.argsort(src_all, kind="stable")
    enc_sorted = enc_all[order]
    starts = np.searchsorted(src_all[order], np.arange(N + 1))

    imb = np.zeros(2 * N, np.int32)                    # (forced-lo - forced-hi) per list
    core_asg = np.empty(N, np.int64)
    pos_asg = np.empty(N, np.int64)

    for p in range(nown):
        blk = rank_of[NCORES * p:NCORES * p + NCORES]
        rows = 1 * 0 + nown * np.arange(NCORES) + p
        zs = np.where(rows < hi_start, 0, np.where(rows >= lo_end, 2, 1))
        mid_cores = np.where(zs == 1)[0]
        lo_cores = np.where(zs == 0)[0]
        hi_cores = np.where(zs == 2)[0]
        bl = sorted(blk, key=lambda s: -refs[s])
        mids = bl[:len(mid_cores)]
        rest = bl[len(mid_cores):]
        deltas = [imb[enc_sorted[starts[s]:starts[s + 1]]].sum() for s in rest]
        o = np.argsort(deltas)                          # most hi-heavy first -> lo
        for s, c in zip(mids, mid_cores):
            core_asg[s] = c; pos_asg[s] = p
        for i, c in zip(o[:len(lo_cores)], lo_cores):
            s = rest[i]
            core_asg[s] = c; pos_asg[s] = p
            imb[enc_sorted[starts[s]:starts[s + 1]]] += 1
        for i, c in zip(o[len(lo_cores):], hi_cores):
            s = rest[i]
            core_asg[s] = c; pos_asg[s] = p
            imb[enc_sorted[starts[s]:starts[s + 1]]] -= 1
    row = core_asg * nown + pos_asg
    return core_asg, pos_asg, row


def _prep_direction(ei, core_asg, pos_asg, row, n_nodes, nown, nownp, ng,
                    hi_start, lo_end, avg_log):
    """Per-direction index/scal tables (all cores) + shared K schedule."""
    src = np.asarray(ei[0], dtype=np.int64)
    dst = np.asarray(ei[1], dtype=np.int64)
    r = row[src]
    can_lo = r < lo_end
    can_hi = r >= hi_start
    forced_lo = can_lo & ~can_hi
    forced_hi = can_hi & ~can_lo

    percore = []
    KL = np.ones(ng, np.int64)
    KH = np.ones(ng, np.int64)
    for c in range(NCORES):
        sel = core_asg[dst] == c
        pn = pos_asg[dst[sel]]
        rs = r[sel]
        flo_m = forced_lo[sel]
        fhi_m = forced_hi[sel]
        cnt = np.bincount(pn, minlength=nownp)
        flo = np.bincount(pn[flo_m], minlength=nownp)
        fhi = np.bincount(pn[fhi_m], minlength=nownp)
        clo = np.maximum(flo, np.minimum(cnt - fhi, (cnt + 1) // 2))
        chi = cnt - clo
        KL = np.maximum(KL, clo.reshape(ng, P).max(1))
        KH = np.maximum(KH, chi.reshape(ng, P).max(1))
        # per-node neighbor rows, lo-assigned first:
        # sort edges by (pos, flexclass) where flexclass: forced_lo=0, flex=1, forced_hi=2
        fcls = np.where(flo_m, 0, np.where(fhi_m, 2, 1))
        o = np.lexsort((fcls, pn))
        percore.append(dict(pn=pn[o], rs=rs[o], cnt=cnt, clo=clo, chi=chi,
                            starts=np.searchsorted(pn[o], np.arange(nownp + 1))))
    empty_flag = np.zeros(ng, bool)
    for c in range(NCORES):
        cc = percore[c]
        ef = ((cc["clo"] == 0) | (cc["chi"] == 0)).reshape(ng, P).any(1)
        empty_flag |= ef

    out_cores = []
    for c in range(NCORES):
        cc = percore[c]
        cnt, clo, chi, starts = cc["cnt"], cc["clo"], cc["chi"], cc["starts"]
        rs = cc["rs"]
        idx_lo_parts = []
        idx_hi_parts = []
        scal = np.zeros((ng, P, 16), np.float32)
        for g in range(ng):
            kl, kh = int(KL[g]), int(KH[g])
            slo = np.zeros((kl, P), np.int64)
            shi = np.zeros((kh, P), np.int64)
            for p_ in range(P):
                j = g * P + p_
                lst = rs[starts[j]:starts[j + 1]]
                nl, nh = int(clo[j]), int(chi[j])
                lo_rows = lst[:nl]
                hi_rows = lst[nl:]
                if nl > 0:
                    slo[:nl, p_] = lo_rows
                    slo[nl:, p_] = lo_rows[0]
                # else stays 0 (row 0, corrected via npadlo)
                if nh > 0:
                    shi[:nh, p_] = hi_rows - hi_start
                    shi[nh:, p_] = hi_rows[0] - hi_start
                # else stays 0 (row hi_start, corrected via npadhi)
                cj = int(cnt[j])
                safe = max(cj, 1)
                logd = np.log(safe + 1.0)
                scal[g, p_, 0] = -(kl - nl)
                scal[g, p_, 1] = -(kh - nh)
                scal[g, p_, 2] = 0.0 if nl > 0 else BIG
                scal[g, p_, 3] = 0.0 if nh > 0 else BIG
                scal[g, p_, 4] = 1.0 / safe
                scal[g, p_, 5] = 1.0 if cj > 0 else 0.0
                scal[g, p_, 6] = logd / avg_log
                scal[g, p_, 7] = avg_log / logd
                scal[g, p_, 8] = float(cj)
                scal[g, p_, 9] = 1.0 if cj > 1 else 0.0   # varmask
            idx_lo_parts.append(_wrap16(slo.reshape(-1).astype(np.int16)))
            idx_hi_parts.append(_wrap16(shi.reshape(-1).astype(np.int16)))
        out_cores.append(dict(
            idx_lo=np.concatenate(idx_lo_parts, axis=1),
            idx_hi=np.concatenate(idx_hi_parts, axis=1),
            scal=scal))
    return out_cores, KL, KH, empty_flag


def _pack_weights(inputs):
    """Packed weight arrays shared by all cores."""
    w = {}
    acatw = np.zeros((LAYERS, 2, F, 192), np.float32)
    acatb = np.zeros((LAYERS, 2, P, 192), np.float32)
    ppw = np.zeros((LAYERS, 2, F, 5 * 192), np.float32)
    linw = np.zeros((LAYERS, 2, 64, 64), np.float32)
    linb = np.zeros((LAYERS, 2, P, 64), np.float32)
    wbot = np.zeros((LAYERS, 2, F, F), np.float32)
    for l in range(LAYERS):
        for d, tag in enumerate(("in", "out")):
            preW = np.asarray(inputs[f"{tag}_pre_W"][l], np.float32)
            preB = np.asarray(inputs[f"{tag}_pre_b"][l], np.float32)
            postW = np.asarray(inputs[f"{tag}_post_W"][l], np.float32)
            postB = np.asarray(inputs[f"{tag}_post_b"][l], np.float32)
            acatw[l, d] = np.concatenate([preW[0:F], postW[0:F]], axis=1)
            acatb[l, d] = np.tile(np.concatenate([preB, postB])[None, :], (P, 1))
            pp = np.zeros((F, 5 * 192), np.float32)
            for p_ in range(5):
                for k in range(3):
                    rows = postW[F + k * 5 * F + p_ * F: F + k * 5 * F + (p_ + 1) * F]
                    pp[:, p_ * 192 + k * 64: p_ * 192 + (k + 1) * 64] = rows
            ppw[l, d] = pp
            linw[l, d] = np.asarray(inputs[f"{tag}_lin_W"][l], np.float32)
            linb[l, d] = np.tile(np.asarray(inputs[f"{tag}_lin_b"][l], np.float32)[None, :], (P, 1))
            wbot[l, d] = preW[F:2 * F]
    w["acatw"] = acatw; w["acatb"] = acatb; w["ppw"] = ppw
    w["linw"] = linw; w["linb"] = linb; w["wbot"] = wbot
    combW = np.asarray(inputs["comb_W"], np.float32)       # [L, 256, 128]
    w["combw1"] = combW[:, 0:F, :].copy()
    w["combw2"] = combW[:, F:256, :].copy()
    w["combb"] = np.stack([np.tile(np.asarray(inputs["comb_b"][l], np.float32)[None, :], (P, 1))
                           for l in range(LAYERS)])
    w["headw"] = np.asarray(inputs["head_W"], np.float32)
    w["headb"] = np.tile(np.asarray(inputs["head_b"], np.float32)[None, :], (P, 1))
    return w


# -------------------------------------------------------------- device build

def build_program(meta):
    n_nodes = meta["n_nodes"]
    nown = meta["nown"]
    nownp = meta["nownp"]
    ng = meta["ng"]
    hi_start = meta["hi_start"]
    lo_end = meta["lo_end"]
    KL = meta["KL"]          # dict d -> [ng]
    KH = meta["KH"]
    empty_flag = meta["empty_flag"]  # dict d -> [ng] bool
    sum_kl = {d: int(KL[d].sum()) for d in (0, 1)}
    sum_kh = {d: int(KH[d].sum()) for d in (0, 1)}
    maxslots = max(int((KL[d] + KH[d]).max()) for d in (0, 1))

    nc = bacc.Bacc("TRN2", target_bir_lowering=False, debug=False,
                   num_devices=NCORES)

    # ---- DRAM I/O
    T0 = nc.dram_tensor("T0", [n_nodes, 256], FP32, kind="ExternalInput")
    A0_t = nc.dram_tensor("A0", [2, nownp, 192], FP32, kind="ExternalInput")
    xTown_t = nc.dram_tensor("xTown", [P, nownp], FP32, kind="ExternalInput")
    idx_lo = {d: nc.dram_tensor(f"idx_lo{d}", [P, sum_kl[d] * 8], I16, kind="ExternalInput") for d in (0, 1)}
    idx_hi = {d: nc.dram_tensor(f"idx_hi{d}", [P, sum_kh[d] * 8], I16, kind="ExternalInput") for d in (0, 1)}
    scal_t = nc.dram_tensor("scal", [2, ng, P, 16], FP32, kind="ExternalInput")
    acatw_t = nc.dram_tensor("acatw", [LAYERS, 2, F, 192], FP32, kind="ExternalInput")
    acatb_t = nc.dram_tensor("acatb", [LAYERS, 2, P, 192], FP32, kind="ExternalInput")
    ppw_t = nc.dram_tensor("ppw", [LAYERS, 2, F, 5 * 192], FP32, kind="ExternalInput")
    linw_t = nc.dram_tensor("linw", [LAYERS, 2, 64, 64], FP32, kind="ExternalInput")
    linb_t = nc.dram_tensor("linb", [LAYERS, 2, P, 64], FP32, kind="ExternalInput")
    wbot_t = nc.dram_tensor("wbot", [LAYERS, 2, F, F], FP32, kind="ExternalInput")
    combw1_t = nc.dram_tensor("combw1", [LAYERS, F, F], FP32, kind="ExternalInput")
    combw2_t = nc.dram_tensor("combw2", [LAYERS, F, F], FP32, kind="ExternalInput")
    combb_t = nc.dram_tensor("combb", [LAYERS, P, F], FP32, kind="ExternalInput")
    headw_t = nc.dram_tensor("headw", [F, 8], FP32, kind="ExternalInput")
    headb_t = nc.dram_tensor("headb", [P, 8], FP32, kind="ExternalInput")
    out_t = nc.dram_tensor("out", [nown, 8], FP32, kind="ExternalOutput")

    # ---- DRAM internal
    T1own = nc.dram_tensor("T1own", [nown, 256], FP32)
    T1 = nc.dram_tensor("T1", [n_nodes, 256], FP32, addr_space="Shared")

    AF = mybir.ActivationFunctionType
    OP = mybir.AluOpType
    AX = mybir.AxisListType

    with TileContext(nc) as tc:
        with tc.tile_pool(name="const", bufs=1) as constp, \
             tc.tile_pool(name="wts", bufs=1) as wtsp, \
             tc.tile_pool(name="x2T", bufs=1) as x2tp, \
             tc.tile_pool(name="ip", bufs=2) as ipool, \
             tc.tile_pool(name="ap", bufs=2) as apool, \
             tc.tile_pool(name="gt", bufs=2) as gathp, \
             tc.tile_pool(name="sq", bufs=1) as sqp, \
             tc.tile_pool(name="nl", bufs=1) as nlp, \
             tc.tile_pool(name="lt", bufs=2) as ltp, \
             tc.tile_pool(name="xp", bufs=2) as xtp, \
             tc.tile_pool(name="ps", bufs=2, space="PSUM") as psp, \
             tc.tile_pool(name="ps1", bufs=1, space="PSUM") as ps1p, \
             tc.tile_pool(name="psa", bufs=2, space="PSUM") as psap:

            ident = constp.tile([P, P], FP32)
            make_identity(nc, ident[:])
            eps_col = constp.tile([P, 1], FP32)
            nc.vector.memset(eps_col[:], EPS)

            def load_w(dram_ap, shape, tag):
                t = wtsp.tile(shape, FP32, tag=tag)
                nc.sync.dma_start(out=t[:], in_=dram_ap)
                return t

            headw_sb = load_w(headw_t[:], [F, 8], "headw")
            headb_sb = load_w(headb_t[:], [P, 8], "headb")
            acatw_sb = {}
            acatb_sb = {}
            ppw_sb = {}
            linw_sb = {}
            linb_sb = {}
            wbot_sb = {}
            combw1_sb = {}
            combw2_sb = {}
            combb_sb = {}
            for l in range(LAYERS):
                combw1_sb[l] = load_w(combw1_t[l], [F, F], f"combw1_{l}")
                combw2_sb[l] = load_w(combw2_t[l], [F, F], f"combw2_{l}")
                combb_sb[l] = load_w(combb_t[l], [P, F], f"combb_{l}")
                for d in (0, 1):
                    ppw_sb[(l, d)] = load_w(ppw_t[l, d], [F, 5 * 192], f"ppw{l}{d}")
                    linw_sb[(l, d)] = load_w(linw_t[l, d], [64, 64], f"linw{l}{d}")
                    linb_sb[(l, d)] = load_w(linb_t[l, d], [P, 64], f"linb{l}{d}")
                    if l == 1:
                        acatw_sb[(l, d)] = load_w(acatw_t[l, d], [F, 192], f"acatw{l}{d}")
                        acatb_sb[(l, d)] = load_w(acatb_t[l, d], [P, 192], f"acatb{l}{d}")
                    if l == 1:
                        wbot_sb[(l, d)] = load_w(wbot_t[l, d], [F, F], f"wbot{l}{d}")

            # persistent x feature table (transposed); layer 0 reads input x,
            # layer-0 comb overwrites columns with x2 for layer 1.
            x2T = x2tp.tile([P, nownp], FP32)
            nc.sync.dma_start(out=x2T[:], in_=xTown_t[:])

            def stats_dir(l, d, g, Ag, X0g, gt, sc, KLg, KHg, has_empty):
                """Aggregate + post + lin for one (group, dir). Returns h tile [P,64]."""
                v = nc.vector
                W = (KLg + KHg) * F

                def nlt(tag):
                    return nlp.tile([P, F], FP32, tag=tag, name=tag)

                sq = sqp.tile([P, maxslots * F], FP32, tag="sq")
                nc.scalar.activation(sq[:, 0:W], gt[:, 0:W], AF.Square)

                S_f, Q_f = nlt(f"S_f{d}"), nlt(f"Q_f{d}")
                v.tensor_reduce(S_f[:], gt[:, 0:W].rearrange("p (k f) -> p f k", f=F),
                                axis=AX.X, op=OP.add)
                v.tensor_reduce(Q_f[:], sq[:, 0:W].rearrange("p (k f) -> p f k", f=F),
                                axis=AX.X, op=OP.add)
                MN, MX = nlt(f"MN{d}"), nlt(f"MX{d}")
                npl = sc[:, 0:1]
                nph = sc[:, 1:2]
                mlb = sc[:, 2:3]
                mhb = sc[:, 3:4]
                rcp = sc[:, 4:5]
                nemp = sc[:, 5:6]
                f1 = sc[:, 6:7]
                f2 = sc[:, 7:8]
                cntc = sc[:, 8:9]
                t1, t2 = nlt(f"t1{d}"), nlt(f"t2{d}")
                if has_empty:
                    mnlo, mnhi = nlt(f"mnlo{d}"), nlt(f"mnhi{d}")
                    mxlo, mxhi = nlt(f"mxlo{d}"), nlt(f"mxhi{d}")
                    v.tensor_reduce(mnlo[:], gt[:, 0:KLg * F].rearrange("p (k f) -> p f k", f=F),
                                    axis=AX.X, op=OP.min)
                    v.tensor_reduce(mnhi[:], gt[:, KLg * F:W].rearrange("p (k f) -> p f k", f=F),
                                    axis=AX.X, op=OP.min)
                    v.tensor_reduce(mxlo[:], gt[:, 0:KLg * F].rearrange("p (k f) -> p f k", f=F),
                                    axis=AX.X, op=OP.max)
                    v.tensor_reduce(mxhi[:], gt[:, KLg * F:W].rearrange("p (k f) -> p f k", f=F),
                                    axis=AX.X, op=OP.max)
                    v.tensor_scalar(t1[:], mnlo[:], mlb, None, op0=OP.add)
                    v.tensor_scalar(t2[:], mnhi[:], mhb, None, op0=OP.add)
                    v.tensor_tensor(MN[:], t1[:], t2[:], op=OP.min)
                    v.tensor_scalar(t1[:], mxlo[:], mlb, None, op0=OP.subtract)
                    v.tensor_scalar(t2[:], mxhi[:], mhb, None, op0=OP.subtract)
                    v.tensor_tensor(MX[:], t1[:], t2[:], op=OP.max)
                else:
                    v.tensor_reduce(MN[:], gt[:, 0:W].rearrange("p (k f) -> p f k", f=F),
                                    axis=AX.X, op=OP.min)
                    v.tensor_reduce(MX[:], gt[:, 0:W].rearrange("p (k f) -> p f k", f=F),
                                    axis=AX.X, op=OP.max)

                v0lo = gt[:, 0:F]
                v0hi = gt[:, KLg * F:(KLg + 1) * F]
                q0lo = sq[:, 0:F]
                q0hi = sq[:, KLg * F:(KLg + 1) * F]
                S, Q = nlt(f"S{d}"), nlt(f"Q{d}")
                v.scalar_tensor_tensor(t1[:], v0lo, npl, S_f[:], op0=OP.mult, op1=OP.add)
                v.scalar_tensor_tensor(S[:], v0hi, nph, t1[:], op0=OP.mult, op1=OP.add)
                v.scalar_tensor_tensor(t2[:], q0lo, npl, Q_f[:], op0=OP.mult, op1=OP.add)
                v.scalar_tensor_tensor(Q[:], q0hi, nph, t2[:], op0=OP.mult, op1=OP.add)

                s_full, mean = nlt(f"s_full{d}"), nlt(f"mean{d}")
                meanB, std = nlt(f"meanB{d}"), nlt(f"std{d}")
                mn, mx = nlt(f"mn{d}"), nlt(f"mx{d}")
                v.scalar_tensor_tensor(s_full[:], Ag, cntc, S[:], op0=OP.mult, op1=OP.add)
                nc.scalar.activation(mean[:], s_full[:], AF.Copy, scale=rcp)
                nc.scalar.activation(meanB[:], S[:], AF.Copy, scale=rcp)
                nc.scalar.activation(t1[:], Q[:], AF.Copy, scale=rcp)
                v.tensor_mul(t2[:], meanB[:], meanB[:])
                vr1, vr2 = nlt(f"vr1{d}"), nlt(f"vr2{d}")
                v.tensor_sub(vr1[:], t1[:], t2[:])
                v.tensor_scalar_max(vr2[:], vr1[:], 0.0)
                vmsk = sc[:, 9:10]
                v.tensor_scalar(vr1[:], vr2[:], vmsk, None, op0=OP.mult)
                nc.scalar.activation(std[:], vr1[:], AF.Sqrt, bias=eps_col[:, 0:1])
                v.tensor_add(t1[:], Ag, MN[:])
                v.tensor_scalar(mn[:], t1[:], nemp, None, op0=OP.mult)
                v.tensor_add(t2[:], Ag, MX[:])
                v.tensor_scalar(mx[:], t2[:], nemp, None, op0=OP.mult)

                # post: y = X0 + sum_p sum_k f_k*(part_p @ Wp_k)
                py = psap.tile([P, 192], FP32, tag="py")
                for pi, part in enumerate((mean, s_full, std, mn, mx)):
                    pt = psp.tile([P, P], FP32, tag="ptr")
                    nc.tensor.transpose(out=pt[:], in_=part[:], identity=ident[:])
                    partT = xtp.tile([P, P], FP32, tag="partT")
                    nc.scalar.copy(partT[:], pt[:])
                    nc.tensor.matmul(out=py[:], lhsT=partT[:],
                                     rhs=ppw_sb[(l, d)][:, pi * 192:(pi + 1) * 192],
                                     start=(pi == 0), stop=(pi == 4))
                pys = ltp.tile([P, 192], FP32, tag=f"pys{d}", name="pys")
                nc.scalar.copy(pys[:], py[:])
                yt, y64 = nlt(f"yt{d}"), nlt(f"y64{d}")
                v.scalar_tensor_tensor(yt[:, 0:64], pys[:, 64:128], f1,
                                       pys[:, 0:64], op0=OP.mult, op1=OP.add)
                v.scalar_tensor_tensor(yt[:, 64:128], pys[:, 128:192], f2,
                                       X0g, op0=OP.mult, op1=OP.add)
                v.tensor_add(y64[:, 0:64], yt[:, 0:64], yt[:, 64:128])
                pt = psp.tile([P, P], FP32, tag="ptr")
                nc.tensor.transpose(out=pt[:64, :], in_=y64[:, 0:64], identity=ident[:])
                ylhs = xtp.tile([64, P], FP32, tag="ylhs")
                nc.scalar.copy(ylhs[:], pt[:64, :])
                pz = ps1p.tile([P, 64], FP32, tag="pz")
                nc.tensor.matmul(out=pz[:], lhsT=ylhs[:], rhs=linw_sb[(l, d)][:],
                                 start=True, stop=True)
                zb = nlt(f"zb{d}")
                v.tensor_add(zb[:, 0:64], pz[:], linb_sb[(l, d)][:, 0:64])
                h = ltp.tile([P, 64], FP32, tag=f"h{d}", name=f"h{d}")
                nc.scalar.activation(h[:], zb[:, 0:64], AF.Relu)
                return h

            cum_lo = {d: np.concatenate([[0], np.cumsum(KL[d])]) * 8 for d in (0, 1)}
            cum_hi = {d: np.concatenate([[0], np.cumsum(KH[d])]) * 8 for d in (0, 1)}
            for l in range(LAYERS):
                Tsrc = T0 if l == 0 else T1
                for g in range(ng):
                    rows = min(P, nown - g * P)
                    hs = []
                    for d in (0, 1):
                        KLg, KHg = int(KL[d][g]), int(KH[d][g])
                        W = (KLg + KHg) * F
                        ol, oh = int(cum_lo[d][g]), int(cum_hi[d][g])
                        il = ipool.tile([P, KLg * 8], I16, tag=f"il{d}")
                        nc.sync.dma_start(out=il[:], in_=idx_lo[d][:, ol:ol + KLg * 8])
                        ih = ipool.tile([P, KHg * 8], I16, tag=f"ih{d}")
                        nc.sync.dma_start(out=ih[:], in_=idx_hi[d][:, oh:oh + KHg * 8])
                        sc = ipool.tile([P, 16], FP32, tag=f"sc{d}")
                        nc.sync.dma_start(out=sc[:], in_=scal_t[d, g])
                        gt = gathp.tile([P, maxslots * F], FP32, tag=f"gt{d}")
                        nc.gpsimd.dma_gather(
                            out_ap=gt[:, 0:KLg * F].rearrange("p (k f) -> p k f", f=F),
                            in_ap=Tsrc[0:lo_end, d * F:(d + 1) * F],
                            idxs_ap=il[:], num_idxs=KLg * P, num_idxs_reg=KLg * P,
                            elem_size=F, elem_step=256, single_packet=False)
                        nc.gpsimd.dma_gather(
                            out_ap=gt[:, KLg * F:W].rearrange("p (k f) -> p k f", f=F),
                            in_ap=Tsrc[hi_start:n_nodes, d * F:(d + 1) * F],
                            idxs_ap=ih[:], num_idxs=KHg * P, num_idxs_reg=KHg * P,
                            elem_size=F, elem_step=256, single_packet=False)
                        # A row block
                        if l == 0:
                            at = apool.tile([P, 192], FP32, tag=f"at{d}")
                            nc.sync.dma_start(out=at[:], in_=A0_t[d, g * P:(g + 1) * P, :])
                        else:
                            pa = psap.tile([P, 192], FP32, tag="py")
                            nc.tensor.matmul(out=pa[:], lhsT=x2T[:, g * P:(g + 1) * P],
                                             rhs=acatw_sb[(1, d)][:], start=True, stop=True)
                            at = apool.tile([P, 192], FP32, tag=f"at{d}")
                            nc.vector.tensor_add(at[:], pa[:], acatb_sb[(1, d)][:])
                        h = stats_dir(l, d, g, at[:, 0:F], at[:, F:192], gt, sc,
                                      KLg, KHg, bool(empty_flag[d][g]))
                        hs.append(h)

                    # comb
                    hcatT = xtp.tile([P, P], FP32, tag="hcatT")
                    for d in (0, 1):
                        pt = psp.tile([P, P], FP32, tag="ptr")
                        nc.tensor.transpose(out=pt[:64, :], in_=hs[d][:, 0:64], identity=ident[:])
                        nc.vector.tensor_copy(hcatT[d * 64:(d + 1) * 64, :], pt[:64, :])
                    pc = ps1p.tile([P, P], FP32, tag="pc")
                    nc.tensor.matmul(out=pc[:], lhsT=x2T[:, g * P:(g + 1) * P],
                                     rhs=combw1_sb[l][:], start=True, stop=False)
                    nc.tensor.matmul(out=pc[:], lhsT=hcatT[:], rhs=combw2_sb[l][:],
                                     start=False, stop=True)
                    xn = ltp.tile([P, F], FP32, tag="xn")
                    nc.vector.tensor_add(xn[:], pc[:], combb_sb[l][:])
                    if l == 0:
                        x2 = ltp.tile([P, F], FP32, tag="x2", name="x2")
                        nc.scalar.activation(x2[:], xn[:], AF.Relu)
                        pt = psp.tile([P, P], FP32, tag="ptr")
                        nc.tensor.transpose(out=pt[:], in_=x2[:], identity=ident[:])
                        nc.scalar.copy(x2T[:, g * P:(g + 1) * P], pt[:])
                        bt = xtp.tile([P, 256], FP32, tag="bt")
                        for d in (0, 1):
                            pb = ps1p.tile([P, P], FP32, tag="pb")
                            nc.tensor.matmul(out=pb[:], lhsT=x2T[:, g * P:(g + 1) * P],
                                             rhs=wbot_sb[(1, d)][:], start=True, stop=True)
                            nc.scalar.copy(bt[:, d * F:(d + 1) * F], pb[:])
                        nc.sync.dma_start(out=T1own[g * P:g * P + rows, :],
                                          in_=bt[:rows, :])
                    else:
                        x3 = ltp.tile([P, F], FP32, tag="x2", name="x3")
                        nc.scalar.activation(x3[:], xn[:], AF.Relu)
                        pt = psp.tile([P, P], FP32, tag="ptr")
                        nc.tensor.transpose(out=pt[:], in_=x3[:], identity=ident[:])
                        x3T = xtp.tile([P, P], FP32, tag="x3T")
                        nc.scalar.copy(x3T[:], pt[:])
                        ph = ps1p.tile([P, 8], FP32, tag="pz")
                        nc.tensor.matmul(out=ph[:], lhsT=x3T[:], rhs=headw_sb[:],
                                         start=True, stop=True)
                        ot = ltp.tile([P, 8], FP32, tag="ot")
                        nc.vector.tensor_add(ot[:], ph[:], headb_sb[:])
                        nc.sync.dma_start(out=out_t[g * P:g * P + rows, :],
                                          in_=ot[:rows, :])

                if l == 0:
                    nc.gpsimd.collective_compute(
                        "AllGather", OP.bypass,
                        replica_groups=[list(range(NCORES))],
                        ins=[T1own[:]], outs=[T1[:]])

    nc.finalize()
    return nc


# ----------------------------------------------------------------- kernel()

def _host_prep(inputs):
    x = np.asarray(inputs["x"], np.float32)
    n_nodes, f = x.shape
    assert f == F and n_nodes % NCORES == 0
    nown = n_nodes // NCORES
    nownp = ((nown + P - 1) // P) * P
    ng = nownp // P
    lo_end = min(WIN, n_nodes)
    hi_start = max(n_nodes - WIN, 0)

    avg_in = float(np.asarray(inputs["avg_in"]))
    avg_out = float(np.asarray(inputs["avg_out"]))
    eis = {0: np.asarray(inputs["edge_index_in"], np.int64),
           1: np.asarray(inputs["edge_index_out"], np.int64)}

    core_asg, pos_asg, row = _assign_nodes(eis, n_nodes, nown, hi_start, lo_end)

    prep = {}
    KLd = {}
    KHd = {}
    empty_flag = {}
    for d, avg in ((0, avg_in), (1, avg_out)):
        cores, KL, KH, ef = _prep_direction(
            eis[d], core_asg, pos_asg, row, n_nodes, nown, nownp, ng,
            hi_start, lo_end, avg)
        prep[d] = cores
        KLd[d] = KL
        KHd[d] = KH
        empty_flag[d] = ef

    w = _pack_weights(inputs)

    # node order (natural id) per table row
    inv = np.empty(n_nodes, np.int64)
    inv[row] = np.arange(n_nodes)

    # layer-0 tables
    xperm = x[inv]                                     # [N,128] in row order
    T0 = np.concatenate([xperm @ w["wbot"][0, 0], xperm @ w["wbot"][0, 1]],
                        axis=1).astype(np.float32)     # [N, 256]
    A0 = np.zeros((NCORES, 2, nownp, 192), np.float32)
    xTown = np.zeros((NCORES, P, nownp), np.float32)
    for c in range(NCORES):
        xc = xperm[c * nown:(c + 1) * nown]            # [nown, 128] pos order
        for d in (0, 1):
            A0[c, d, :nown] = xc @ w["acatw"][0, d] + w["acatb"][0, d][0][None, :]
        xTown[c, :, :nown] = xc.T

    meta = dict(n_nodes=n_nodes, nown=nown, nownp=nownp, ng=ng,
                hi_start=hi_start, lo_end=lo_end, KL=KLd, KH=KHd,
                empty_flag=empty_flag)
    return meta, prep, w, T0, A0, xTown, core_asg, pos_asg


def _in_maps(meta, prep, w, T0, A0, xTown):
    maps = []
    for c in range(NCORES):
        m = dict(T0=T0, A0=A0[c], xTown=xTown[c],
                 acatw=w["acatw"], acatb=w["acatb"], ppw=w["ppw"],
                 linw=w["linw"], linb=w["linb"], wbot=w["wbot"],
                 combw1=w["combw1"], combw2=w["combw2"], combb=w["combb"],
                 headw=w["headw"], headb=w["headb"],
                 scal=np.stack([prep[0][c]["scal"], prep[1][c]["scal"]]))
        for d in (0, 1):
            m[f"idx_lo{d}"] = prep[d][c]["idx_lo"]
            m[f"idx_hi{d}"] = prep[d][c]["idx_hi"]
        maps.append(m)
    return maps


def _install_ntff_hook():
    import sys
    import types
    try:
        from antenv.axon_hooks import get_axon_ntff_profile_hook  # noqa: F401
        return
    except ImportError:
        pass
    try:
        mod = types.ModuleType("antenv.axon_hooks")
        hook = {"h": None}
        mod.set_axon_ntff_profile_hook = lambda h: hook.__setitem__("h", h)
        mod.get_axon_ntff_profile_hook = lambda: hook["h"]
        sys.modules["antenv.axon_hooks"] = mod
        import antenv
        antenv.axon_hooks = mod
        from trn_agent_boot.trn_boot import _ntff_profile_via_ctypes
        mod.set_axon_ntff_profile_hook(
            _ntff_profile_via_ctypes("/opt/axon/libaxon_pjrt.so"))
    except Exception:
        pass


def kernel(**inputs):
    meta, prep, w, T0, A0, xTown, core_asg, pos_asg = _host_prep(inputs)
    nc = build_program(meta)
    maps = _in_maps(meta, prep, w, T0, A0, xTown)

    trace = bool(int(os.environ.get("PNA_TRACE", "0")))
    if trace:
        _install_ntff_hook()
    res = run_bass_kernel_spmd(nc, maps, core_ids=list(range(NCORES)),
                               trace=trace)
    if trace and res.exec_time_ns is not None:
        print(f"HW exec time: {res.exec_time_ns} ns")
    n_nodes = meta["n_nodes"]
    nown = meta["nown"]
    out = np.empty((n_nodes, 8), np.float32)
    for c in range(NCORES):
        o = res.results[c]["out"]
        sel = core_asg == c
        out[sel] = o[pos_asg[sel]]
    return out.astype(np.float32)


def kernel_sim(**inputs):
    """MultiCoreSim path for small-input debugging."""
    from concourse.bass_interp import MultiCoreSim
    meta, prep, w, T0, A0, xTown, core_asg, pos_asg = _host_prep(inputs)
    nc = build_program(meta)
    maps = _in_maps(meta, prep, w, T0, A0, xTown)
    sim = MultiCoreSim(nc, num_cores=NCORES, trace=False,
                       require_finite=False, require_nnan=False)
    for c in range(NCORES):
        cs = sim.cores[c]
        for k, vv in maps[c].items():
            cs.tensor(k)[:] = vv
    sim.simulate(check_with_hw=False)
    n_nodes = meta["n_nodes"]
    out = np.empty((n_nodes, 8), np.float32)
    for c in range(NCORES):
        o = np.array(sim.cores[c].tensor("out"))
        sel = core_asg == c
        out[sel] = o[pos_asg[sel]]
    return out.astype(np.float32)


# revision 20
# speedup vs baseline: 2.5343x; 1.2102x over previous
"""DirectedDualPNA on 8 Trainium2 NeuronCores — v2.

Strategy (node-sharded, fused group pipeline):
  m_e = A[dst] + B[src] decomposition (pre-MLP splits over the concat), so per
  edge only the 512B B row is gathered.  All per-node tables live in one fused
  [50000, 256] DRAM table T = [B_in | B_out] in a custom row order:

  - nodes are globally degree-sorted (key = max(deg_in, deg_out)); rank r maps
    to core r%8.  Each core's groups of 128 nodes are degree-homogeneous, so
    the shared SPMD slot schedule K[g] is tight across cores.
  - a gather's int16 index addresses at most 32768 rows, so each group issues
    two gathers: window LO = rows [0, 32768), window HI = rows [N-32768, N).
    Which (core,pos) slot a source node occupies decides its window zone; ties
    in the degree sort are broken to (a) put high-traffic sources in the
    overlap zone (edges become window-flexible) and (b) 2-color the rest to
    balance every destination list's forced lo/hi counts (greedy discrepancy).
  - per group, per direction: gather lo+hi slots, one Square pass (scalar
    engine), then strided tensor_reduce folds (sum/sumsq full-span, min/max
    full-span or per-side when a group contains empty sides), then the PNA
    tail (post/lin) via PE matmuls, comb fused in-loop (no h scatter), and the
    next layer's B rows produced immediately (x2 @ Wbot) into T1own.
  - one AllGather of T1own rows between layers; layer-0 tables (T0, A0) are
    precomputed on the host.  Output rows are un-permuted on the host.
"""

import os
import numpy as np

import concourse.bass as bass
import concourse.mybir as mybir
from concourse import bacc
from concourse.bass_utils import run_bass_kernel_spmd
from concourse.tile import TileContext
from concourse.masks import make_identity

P = 128
F = 128
NCORES = 8
LAYERS = 2
EPS = 1e-5
BIG = 1e30
WIN = 32768
FP32 = mybir.dt.float32
I16 = mybir.dt.int16


# ----------------------------------------------------------------- host prep

def _wrap16(flat):
    """[n] int16 -> wrapped [128, n//16]: position j lives at (j%16, j//16),
    replicated across the 8 Q7 cores (every 16 partitions)."""
    n = flat.shape[0]
    assert n % 16 == 0
    w = flat.reshape(n // 16, 16).T.astype(np.int16)
    return np.tile(w, (8, 1))


def _assign_nodes(eis, n_nodes, nown, hi_start, lo_end):
    """Global degree sort + zone-aware slot assignment.

    Returns core_asg, pos_asg (per natural node) and row (table row)."""
    N = n_nodes
    cin = np.bincount(eis[0][1], minlength=N)
    cout = np.bincount(eis[1][1], minlength=N)
    key = np.maximum(cin, cout)
    rank_of = np.argsort(-key, kind="stable")          # rank i -> node
    refs = np.bincount(eis[0][0], minlength=N) + np.bincount(eis[1][0], minlength=N)

    # per-src edge CSR over (dir, dst) for the greedy coloring
    src_all = np.concatenate([eis[0][0], eis[1][0]])
    enc_all = np.concatenate([eis[0][1], N + eis[1][1]])
    order = np.argsort(src_all, kind="stable")
    enc_sorted = enc_all[order]
    starts = np.searchsorted(src_all[order], np.arange(N + 1))

    imb = np.zeros(2 * N, np.int32)                    # (forced-lo - forced-hi) per list
    core_asg = np.empty(N, np.int64)
    pos_asg = np.empty(N, np.int64)
    colored = np.zeros(N, np.int8)                     # 0 none, 1 lo, 2 hi

    for pass_ in range(2):
        for p in range(nown):
            blk = rank_of[NCORES * p:NCORES * p + NCORES]
            rows = nown * np.arange(NCORES) + p
            zs = np.where(rows < hi_start, 0, np.where(rows >= lo_end, 2, 1))
            mid_cores = np.where(zs == 1)[0]
            lo_cores = np.where(zs == 0)[0]
            hi_cores = np.where(zs == 2)[0]
            bl = sorted(blk, key=lambda s: -refs[s])
            mids = bl[:len(mid_cores)]
            rest = bl[len(mid_cores):]
            if pass_ == 1:
                # undo this block's pass-0 contribution, then re-decide
                for s in rest:
                    e = enc_sorted[starts[s]:starts[s + 1]]
                    if colored[s] == 1:
                        imb[e] -= 1
                    elif colored[s] == 2:
                        imb[e] += 1
            deltas = [imb[enc_sorted[starts[s]:starts[s + 1]]].sum() for s in rest]
            o = np.argsort(deltas)                      # most hi-heavy first -> lo
            for s, c in zip(mids, mid_cores):
                core_asg[s] = c; pos_asg[s] = p
            for i, c in zip(o[:len(lo_cores)], lo_cores):
                s = rest[i]
                core_asg[s] = c; pos_asg[s] = p; colored[s] = 1
                imb[enc_sorted[starts[s]:starts[s + 1]]] += 1
            for i, c in zip(o[len(lo_cores):], hi_cores):
                s = rest[i]
                core_asg[s] = c; pos_asg[s] = p; colored[s] = 2
                imb[enc_sorted[starts[s]:starts[s + 1]]] -= 1
    row = core_asg * nown + pos_asg
    return core_asg, pos_asg, row


def _prep_direction(ei, core_asg, pos_asg, row, n_nodes, nown, nownp, ng,
                    hi_start, lo_end, avg_log):
    """Per-direction index/scal tables (all cores) + shared K schedule."""
    src = np.asarray(ei[0], dtype=np.int64)
    dst = np.asarray(ei[1], dtype=np.int64)
    r = row[src]
    can_lo = r < lo_end
    can_hi = r >= hi_start
    forced_lo = can_lo & ~can_hi
    forced_hi = can_hi & ~can_lo

    percore = []
    KL = np.ones(ng, np.int64)
    KH = np.ones(ng, np.int64)
    for c in range(NCORES):
        sel = core_asg[dst] == c
        pn = pos_asg[dst[sel]]
        rs = r[sel]
        flo_m = forced_lo[sel]
        fhi_m = forced_hi[sel]
        cnt = np.bincount(pn, minlength=nownp)
        flo = np.bincount(pn[flo_m], minlength=nownp)
        fhi = np.bincount(pn[fhi_m], minlength=nownp)
        clo = np.maximum(flo, np.minimum(cnt - fhi, (cnt + 1) // 2))
        chi = cnt - clo
        KL = np.maximum(KL, clo.reshape(ng, P).max(1))
        KH = np.maximum(KH, chi.reshape(ng, P).max(1))
        # per-node neighbor rows, lo-assigned first:
        # sort edges by (pos, flexclass) where flexclass: forced_lo=0, flex=1, forced_hi=2
        fcls = np.where(flo_m, 0, np.where(fhi_m, 2, 1))
        o = np.lexsort((fcls, pn))
        percore.append(dict(pn=pn[o], rs=rs[o], cnt=cnt, clo=clo, chi=chi,
                            starts=np.searchsorted(pn[o], np.arange(nownp + 1))))
    empty_flag = np.zeros(ng, bool)
    for c in range(NCORES):
        cc = percore[c]
        ef = ((cc["clo"] == 0) | (cc["chi"] == 0)).reshape(ng, P).any(1)
        empty_flag |= ef

    out_cores = []
    for c in range(NCORES):
        cc = percore[c]
        cnt, clo, chi, starts = cc["cnt"], cc["clo"], cc["chi"], cc["starts"]
        rs = cc["rs"]
        idx_lo_parts = []
        idx_hi_parts = []
        scal = np.zeros((ng, P, 16), np.float32)
        for g in range(ng):
            kl, kh = int(KL[g]), int(KH[g])
            slo = np.zeros((kl, P), np.int64)
            shi = np.zeros((kh, P), np.int64)
            for p_ in range(P):
                j = g * P + p_
                lst = rs[starts[j]:starts[j + 1]]
                nl, nh = int(clo[j]), int(chi[j])
                lo_rows = lst[:nl]
                hi_rows = lst[nl:]
                if nl > 0:
                    slo[:nl, p_] = lo_rows
                    slo[nl:, p_] = lo_rows[0]
                # else stays 0 (row 0, corrected via npadlo)
                if nh > 0:
                    shi[:nh, p_] = hi_rows - hi_start
                    shi[nh:, p_] = hi_rows[0] - hi_start
                # else stays 0 (row hi_start, corrected via npadhi)
                cj = int(cnt[j])
                safe = max(cj, 1)
                logd = np.log(safe + 1.0)
                scal[g, p_, 0] = -(kl - nl)
                scal[g, p_, 1] = -(kh - nh)
                scal[g, p_, 2] = 0.0 if nl > 0 else BIG
                scal[g, p_, 3] = 0.0 if nh > 0 else BIG
                scal[g, p_, 4] = 1.0 / safe
                scal[g, p_, 5] = 1.0 if cj > 0 else 0.0
                scal[g, p_, 6] = logd / avg_log
                scal[g, p_, 7] = avg_log / logd
                scal[g, p_, 8] = float(cj)
                scal[g, p_, 9] = 1.0 if cj > 1 else 0.0   # varmask
            idx_lo_parts.append(_wrap16(slo.reshape(-1).astype(np.int16)))
            idx_hi_parts.append(_wrap16(shi.reshape(-1).astype(np.int16)))
        out_cores.append(dict(
            idx_lo=np.concatenate(idx_lo_parts, axis=1),
            idx_hi=np.concatenate(idx_hi_parts, axis=1),
            scal=scal))
    return out_cores, KL, KH, empty_flag


def _pack_weights(inputs):
    """Packed weight arrays shared by all cores."""
    w = {}
    acatw = np.zeros((LAYERS, 2, F, 192), np.float32)
    acatb = np.zeros((LAYERS, 2, P, 192), np.float32)
    ppw = np.zeros((LAYERS, 2, F, 5 * 192), np.float32)
    linw = np.zeros((LAYERS, 2, 64, 64), np.float32)
    linb = np.zeros((LAYERS, 2, P, 64), np.float32)
    wbot = np.zeros((LAYERS, 2, F, F), np.float32)
    for l in range(LAYERS):
        for d, tag in enumerate(("in", "out")):
            preW = np.asarray(inputs[f"{tag}_pre_W"][l], np.float32)
            preB = np.asarray(inputs[f"{tag}_pre_b"][l], np.float32)
            postW = np.asarray(inputs[f"{tag}_post_W"][l], np.float32)
            postB = np.asarray(inputs[f"{tag}_post_b"][l], np.float32)
            acatw[l, d] = np.concatenate([preW[0:F], postW[0:F]], axis=1)
            acatb[l, d] = np.tile(np.concatenate([preB, postB])[None, :], (P, 1))
            pp = np.zeros((F, 5 * 192), np.float32)
            for p_ in range(5):
                for k in range(3):
                    rows = postW[F + k * 5 * F + p_ * F: F + k * 5 * F + (p_ + 1) * F]
                    pp[:, p_ * 192 + k * 64: p_ * 192 + (k + 1) * 64] = rows
            ppw[l, d] = pp
            linw[l, d] = np.asarray(inputs[f"{tag}_lin_W"][l], np.float32)
            linb[l, d] = np.tile(np.asarray(inputs[f"{tag}_lin_b"][l], np.float32)[None, :], (P, 1))
            wbot[l, d] = preW[F:2 * F]
    w["acatw"] = acatw; w["acatb"] = acatb; w["ppw"] = ppw
    w["linw"] = linw; w["linb"] = linb; w["wbot"] = wbot
    combW = np.asarray(inputs["comb_W"], np.float32)       # [L, 256, 128]
    w["combw1"] = combW[:, 0:F, :].copy()
    w["combw2"] = combW[:, F:256, :].copy()
    w["combb"] = np.stack([np.tile(np.asarray(inputs["comb_b"][l], np.float32)[None, :], (P, 1))
                           for l in range(LAYERS)])
    w["headw"] = np.asarray(inputs["head_W"], np.float32)
    w["headb"] = np.tile(np.asarray(inputs["head_b"], np.float32)[None, :], (P, 1))
    return w


# -------------------------------------------------------------- device build

def build_program(meta):
    n_nodes = meta["n_nodes"]
    nown = meta["nown"]
    nownp = meta["nownp"]
    ng = meta["ng"]
    hi_start = meta["hi_start"]
    lo_end = meta["lo_end"]
    KL = meta["KL"]          # dict d -> [ng]
    KH = meta["KH"]
    empty_flag = meta["empty_flag"]  # dict d -> [ng] bool
    sum_kl = {d: int(KL[d].sum()) for d in (0, 1)}
    sum_kh = {d: int(KH[d].sum()) for d in (0, 1)}
    maxslots = max(int((KL[d] + KH[d]).max()) for d in (0, 1))

    nc = bacc.Bacc("TRN2", target_bir_lowering=False, debug=False,
                   num_devices=NCORES)

    # ---- DRAM I/O
    T0 = nc.dram_tensor("T0", [n_nodes, 256], FP32, kind="ExternalInput")
    A0_t = nc.dram_tensor("A0", [2, nownp, 192], FP32, kind="ExternalInput")
    xTown_t = nc.dram_tensor("xTown", [P, nownp], FP32, kind="ExternalInput")
    idx_lo = {d: nc.dram_tensor(f"idx_lo{d}", [P, sum_kl[d] * 8], I16, kind="ExternalInput") for d in (0, 1)}
    idx_hi = {d: nc.dram_tensor(f"idx_hi{d}", [P, sum_kh[d] * 8], I16, kind="ExternalInput") for d in (0, 1)}
    scal_t = nc.dram_tensor("scal", [2, ng, P, 16], FP32, kind="ExternalInput")
    acatw_t = nc.dram_tensor("acatw", [LAYERS, 2, F, 192], FP32, kind="ExternalInput")
    acatb_t = nc.dram_tensor("acatb", [LAYERS, 2, P, 192], FP32, kind="ExternalInput")
    ppw_t = nc.dram_tensor("ppw", [LAYERS, 2, F, 5 * 192], FP32, kind="ExternalInput")
    linw_t = nc.dram_tensor("linw", [LAYERS, 2, 64, 64], FP32, kind="ExternalInput")
    linb_t = nc.dram_tensor("linb", [LAYERS, 2, P, 64], FP32, kind="ExternalInput")
    wbot_t = nc.dram_tensor("wbot", [LAYERS, 2, F, F], FP32, kind="ExternalInput")
    combw1_t = nc.dram_tensor("combw1", [LAYERS, F, F], FP32, kind="ExternalInput")
    combw2_t = nc.dram_tensor("combw2", [LAYERS, F, F], FP32, kind="ExternalInput")
    combb_t = nc.dram_tensor("combb", [LAYERS, P, F], FP32, kind="ExternalInput")
    headw_t = nc.dram_tensor("headw", [F, 8], FP32, kind="ExternalInput")
    headb_t = nc.dram_tensor("headb", [P, 8], FP32, kind="ExternalInput")
    out_t = nc.dram_tensor("out", [nown, 8], FP32, kind="ExternalOutput")

    # ---- DRAM internal
    T1own = nc.dram_tensor("T1own", [nown, 256], FP32)
    T1 = nc.dram_tensor("T1", [n_nodes, 256], FP32, addr_space="Shared")

    AF = mybir.ActivationFunctionType
    OP = mybir.AluOpType
    AX = mybir.AxisListType

    with TileContext(nc) as tc:
        with tc.tile_pool(name="const", bufs=1) as constp, \
             tc.tile_pool(name="wts", bufs=1) as wtsp, \
             tc.tile_pool(name="x2T", bufs=1) as x2tp, \
             tc.tile_pool(name="ip", bufs=2) as ipool, \
             tc.tile_pool(name="ap", bufs=2) as apool, \
             tc.tile_pool(name="gt", bufs=2) as gathp, \
             tc.tile_pool(name="sq", bufs=1) as sqp, \
             tc.tile_pool(name="nl", bufs=1) as nlp, \
             tc.tile_pool(name="lt", bufs=2) as ltp, \
             tc.tile_pool(name="xp", bufs=2) as xtp, \
             tc.tile_pool(name="ps", bufs=2, space="PSUM") as psp, \
             tc.tile_pool(name="ps1", bufs=1, space="PSUM") as ps1p, \
             tc.tile_pool(name="psa", bufs=2, space="PSUM") as psap:

            ident = constp.tile([P, P], FP32)
            make_identity(nc, ident[:])
            eps_col = constp.tile([P, 1], FP32)
            nc.vector.memset(eps_col[:], EPS)
            zeroF = constp.tile([P, F], FP32)
            nc.vector.memset(zeroF[:], 0.0)

            def load_w(dram_ap, shape, tag):
                t = wtsp.tile(shape, FP32, tag=tag)
                nc.sync.dma_start(out=t[:], in_=dram_ap)
                return t

            headw_sb = load_w(headw_t[:], [F, 8], "headw")
            headb_sb = load_w(headb_t[:], [P, 8], "headb")
            acatw_sb = {}
            acatb_sb = {}
            ppw_sb = {}
            linw_sb = {}
            linb_sb = {}
            wbot_sb = {}
            combw1_sb = {}
            combw2_sb = {}
            combb_sb = {}
            for l in range(LAYERS):
                combw1_sb[l] = load_w(combw1_t[l], [F, F], f"combw1_{l}")
                combw2_sb[l] = load_w(combw2_t[l], [F, F], f"combw2_{l}")
                combb_sb[l] = load_w(combb_t[l], [P, F], f"combb_{l}")
                for d in (0, 1):
                    ppw_sb[(l, d)] = load_w(ppw_t[l, d], [F, 5 * 192], f"ppw{l}{d}")
                    linw_sb[(l, d)] = load_w(linw_t[l, d], [64, 64], f"linw{l}{d}")
                    linb_sb[(l, d)] = load_w(linb_t[l, d], [P, 64], f"linb{l}{d}")
                    if l == 1:
                        acatw_sb[(l, d)] = load_w(acatw_t[l, d], [F, 192], f"acatw{l}{d}")
                        acatb_sb[(l, d)] = load_w(acatb_t[l, d], [P, 192], f"acatb{l}{d}")
                    if l == 1:
                        wbot_sb[(l, d)] = load_w(wbot_t[l, d], [F, F], f"wbot{l}{d}")

            # persistent x feature table (transposed); layer 0 reads input x,
            # layer-0 comb overwrites columns with x2 for layer 1.
            x2T = x2tp.tile([P, nownp], FP32)
            nc.sync.dma_start(out=x2T[:], in_=xTown_t[:])

            def stats_dir(l, d, g, Ag, X0g, gt, sc, KLg, KHg, has_empty):
                """Aggregate + post + lin for one (group, dir). Returns h tile [P,64]."""
                v = nc.vector
                W = (KLg + KHg) * F

                def nlt(tag):
                    return nlp.tile([P, F], FP32, tag=tag, name=tag)

                sq = sqp.tile([P, maxslots * F], FP32, tag="sq")
                nc.scalar.activation(sq[:, 0:W], gt[:, 0:W], AF.Square)

                S_f, Q_f = nlt(f"S_f{d}"), nlt(f"Q_f{d}")
                v.tensor_reduce(S_f[:], gt[:, 0:W].rearrange("p (k f) -> p f k", f=F),
                                axis=AX.X, op=OP.add)
                v.tensor_reduce(Q_f[:], sq[:, 0:W].rearrange("p (k f) -> p f k", f=F),
                                axis=AX.X, op=OP.add)
                MN, MX = nlt(f"MN{d}"), nlt(f"MX{d}")
                npl = sc[:, 0:1]
                nph = sc[:, 1:2]
                mlb = sc[:, 2:3]
                mhb = sc[:, 3:4]
                rcp = sc[:, 4:5]
                nemp = sc[:, 5:6]
                f1 = sc[:, 6:7]
                f2 = sc[:, 7:8]
                cntc = sc[:, 8:9]
                t1, t2 = nlt(f"t1{d}"), nlt(f"t2{d}")
                if has_empty:
                    mnlo, mnhi = nlt(f"mnlo{d}"), nlt(f"mnhi{d}")
                    mxlo, mxhi = nlt(f"mxlo{d}"), nlt(f"mxhi{d}")
                    v.tensor_reduce(mnlo[:], gt[:, 0:KLg * F].rearrange("p (k f) -> p f k", f=F),
                                    axis=AX.X, op=OP.min)
                    v.tensor_reduce(mnhi[:], gt[:, KLg * F:W].rearrange("p (k f) -> p f k", f=F),
                                    axis=AX.X, op=OP.min)
                    v.tensor_reduce(mxlo[:], gt[:, 0:KLg * F].rearrange("p (k f) -> p f k", f=F),
                                    axis=AX.X, op=OP.max)
                    v.tensor_reduce(mxhi[:], gt[:, KLg * F:W].rearrange("p (k f) -> p f k", f=F),
                                    axis=AX.X, op=OP.max)
                    v.scalar_tensor_tensor(t1[:], mnlo[:], mlb, zeroF[:], op0=OP.add, op1=OP.add)
                    v.scalar_tensor_tensor(t2[:], mnhi[:], mhb, zeroF[:], op0=OP.add, op1=OP.add)
                    v.tensor_tensor(MN[:], t1[:], t2[:], op=OP.min)
                    v.scalar_tensor_tensor(t1[:], mxlo[:], mlb, zeroF[:], op0=OP.subtract, op1=OP.add)
                    v.scalar_tensor_tensor(t2[:], mxhi[:], mhb, zeroF[:], op0=OP.subtract, op1=OP.add)
                    v.tensor_tensor(MX[:], t1[:], t2[:], op=OP.max)
                else:
                    v.tensor_reduce(MN[:], gt[:, 0:W].rearrange("p (k f) -> p f k", f=F),
                                    axis=AX.X, op=OP.min)
                    v.tensor_reduce(MX[:], gt[:, 0:W].rearrange("p (k f) -> p f k", f=F),
                                    axis=AX.X, op=OP.max)

                v0lo = gt[:, 0:F]
                v0hi = gt[:, KLg * F:(KLg + 1) * F]
                q0lo = sq[:, 0:F]
                q0hi = sq[:, KLg * F:(KLg + 1) * F]
                S, Q = nlt(f"S{d}"), nlt(f"Q{d}")
                v.scalar_tensor_tensor(t1[:], v0lo, npl, S_f[:], op0=OP.mult, op1=OP.add)
                v.scalar_tensor_tensor(S[:], v0hi, nph, t1[:], op0=OP.mult, op1=OP.add)
                v.scalar_tensor_tensor(t2[:], q0lo, npl, Q_f[:], op0=OP.mult, op1=OP.add)
                v.scalar_tensor_tensor(Q[:], q0hi, nph, t2[:], op0=OP.mult, op1=OP.add)

                s_full, mean = nlt(f"s_full{d}"), nlt(f"mean{d}")
                meanB, std = nlt(f"meanB{d}"), nlt(f"std{d}")
                mn, mx = nlt(f"mn{d}"), nlt(f"mx{d}")
                v.scalar_tensor_tensor(s_full[:], Ag, cntc, S[:], op0=OP.mult, op1=OP.add)
                nc.scalar.activation(mean[:], s_full[:], AF.Copy, scale=rcp)
                nc.scalar.activation(meanB[:], S[:], AF.Copy, scale=rcp)
                nc.scalar.activation(t1[:], Q[:], AF.Copy, scale=rcp)
                nc.scalar.activation(t2[:], meanB[:], AF.Square)
                vr1, vr2 = nlt(f"vr1{d}"), nlt(f"vr2{d}")
                v.tensor_sub(vr1[:], t1[:], t2[:])
                v.scalar_tensor_tensor(vr2[:], vr1[:], 0.0, zeroF[:], op0=OP.max, op1=OP.add)
                vmsk = sc[:, 9:10]
                nc.scalar.activation(std[:], vr2[:], AF.Sqrt, scale=vmsk, bias=eps_col[:, 0:1])
                if has_empty:
                    Agn = nlt(f"Agn{d}")
                    v.scalar_tensor_tensor(Agn[:], Ag, nemp, zeroF[:], op0=OP.mult, op1=OP.add)
                    v.scalar_tensor_tensor(mn[:], MN[:], nemp, Agn[:], op0=OP.mult, op1=OP.add)
                    v.scalar_tensor_tensor(mx[:], MX[:], nemp, Agn[:], op0=OP.mult, op1=OP.add)
                else:
                    v.tensor_add(mn[:], Ag, MN[:])
                    v.tensor_add(mx[:], Ag, MX[:])

                # post: y = X0 + sum_p sum_k f_k*(part_p @ Wp_k)
                py = psap.tile([P, 192], FP32, tag="py")
                for pi, part in enumerate((mean, s_full, std, mn, mx)):
                    pt = psp.tile([P, P], FP32, tag="ptr")
                    nc.tensor.transpose(out=pt[:], in_=part[:], identity=ident[:])
                    partT = xtp.tile([P, P], FP32, tag="partT")
                    nc.scalar.copy(partT[:], pt[:])
                    nc.tensor.matmul(out=py[:], lhsT=partT[:],
                                     rhs=ppw_sb[(l, d)][:, pi * 192:(pi + 1) * 192],
                                     start=(pi == 0), stop=(pi == 4))
                pys = ltp.tile([P, 192], FP32, tag=f"pys{d}", name="pys")
                nc.scalar.copy(pys[:], py[:])
                yt, y64 = nlt(f"yt{d}"), nlt(f"y64{d}")
                v.scalar_tensor_tensor(yt[:, 0:64], pys[:, 64:128], f1,
                                       pys[:, 0:64], op0=OP.mult, op1=OP.add)
                v.scalar_tensor_tensor(yt[:, 64:128], pys[:, 128:192], f2,
                                       X0g, op0=OP.mult, op1=OP.add)
                v.tensor_add(y64[:, 0:64], yt[:, 0:64], yt[:, 64:128])
                pt = psp.tile([P, P], FP32, tag="ptr")
                nc.tensor.transpose(out=pt[:64, :], in_=y64[:, 0:64], identity=ident[:])
                ylhs = xtp.tile([64, P], FP32, tag="ylhs")
                nc.scalar.copy(ylhs[:], pt[:64, :])
                pz = ps1p.tile([P, 64], FP32, tag="pz")
                nc.tensor.matmul(out=pz[:], lhsT=ylhs[:], rhs=linw_sb[(l, d)][:],
                                 start=True, stop=True)
                zb = nlt(f"zb{d}")
                v.tensor_add(zb[:, 0:64], pz[:], linb_sb[(l, d)][:, 0:64])
                h = ltp.tile([P, 64], FP32, tag=f"h{d}", name=f"h{d}")
                nc.scalar.activation(h[:], zb[:, 0:64], AF.Relu)
                return h

            cum_lo = {d: np.concatenate([[0], np.cumsum(KL[d])]) * 8 for d in (0, 1)}
            cum_hi = {d: np.concatenate([[0], np.cumsum(KH[d])]) * 8 for d in (0, 1)}
            for l in range(LAYERS):
                Tsrc = T0 if l == 0 else T1
                for g in range(ng):
                    rows = min(P, nown - g * P)
                    hs = []
                    for d in (0, 1):
                        KLg, KHg = int(KL[d][g]), int(KH[d][g])
                        W = (KLg + KHg) * F
                        ol, oh = int(cum_lo[d][g]), int(cum_hi[d][g])
                        il = ipool.tile([P, KLg * 8], I16, tag=f"il{d}")
                        nc.sync.dma_start(out=il[:], in_=idx_lo[d][:, ol:ol + KLg * 8])
                        ih = ipool.tile([P, KHg * 8], I16, tag=f"ih{d}")
                        nc.sync.dma_start(out=ih[:], in_=idx_hi[d][:, oh:oh + KHg * 8])
                        sc = ipool.tile([P, 16], FP32, tag=f"sc{d}")
                        nc.sync.dma_start(out=sc[:], in_=scal_t[d, g])
                        gt = gathp.tile([P, maxslots * F], FP32, tag=f"gt{d}")
                        nc.gpsimd.dma_gather(
                            out_ap=gt[:, 0:KLg * F].rearrange("p (k f) -> p k f", f=F),
                            in_ap=Tsrc[0:lo_end, d * F:(d + 1) * F],
                            idxs_ap=il[:], num_idxs=KLg * P, num_idxs_reg=KLg * P,
                            elem_size=F, elem_step=256, single_packet=False)
                        nc.gpsimd.dma_gather(
                            out_ap=gt[:, KLg * F:W].rearrange("p (k f) -> p k f", f=F),
                            in_ap=Tsrc[hi_start:n_nodes, d * F:(d + 1) * F],
                            idxs_ap=ih[:], num_idxs=KHg * P, num_idxs_reg=KHg * P,
                            elem_size=F, elem_step=256, single_packet=False)
                        # A row block
                        if l == 0:
                            at = apool.tile([P, 192], FP32, tag=f"at{d}")
                            nc.sync.dma_start(out=at[:], in_=A0_t[d, g * P:(g + 1) * P, :])
                        else:
                            pa = psap.tile([P, 192], FP32, tag="py")
                            nc.tensor.matmul(out=pa[:], lhsT=x2T[:, g * P:(g + 1) * P],
                                             rhs=acatw_sb[(1, d)][:], start=True, stop=True)
                            at = apool.tile([P, 192], FP32, tag=f"at{d}")
                            nc.vector.tensor_add(at[:], pa[:], acatb_sb[(1, d)][:])
                        h = stats_dir(l, d, g, at[:, 0:F], at[:, F:192], gt, sc,
                                      KLg, KHg, bool(empty_flag[d][g]))
                        hs.append(h)

                    # comb
                    hcatT = xtp.tile([P, P], FP32, tag="hcatT")
                    for d in (0, 1):
                        pt = psp.tile([P, P], FP32, tag="ptr")
                        nc.tensor.transpose(out=pt[:64, :], in_=hs[d][:, 0:64], identity=ident[:])
                        nc.vector.tensor_copy(hcatT[d * 64:(d + 1) * 64, :], pt[:64, :])
                    pc = ps1p.tile([P, P], FP32, tag="pc")
                    nc.tensor.matmul(out=pc[:], lhsT=x2T[:, g * P:(g + 1) * P],
                                     rhs=combw1_sb[l][:], start=True, stop=False)
                    nc.tensor.matmul(out=pc[:], lhsT=hcatT[:], rhs=combw2_sb[l][:],
                                     start=False, stop=True)
                    xn = ltp.tile([P, F], FP32, tag="xn")
                    nc.vector.tensor_add(xn[:], pc[:], combb_sb[l][:])
                    if l == 0:
                        x2 = ltp.tile([P, F], FP32, tag="x2", name="x2")
                        nc.scalar.activation(x2[:], xn[:], AF.Relu)
                        pt = psp.tile([P, P], FP32, tag="ptr")
                        nc.tensor.transpose(out=pt[:], in_=x2[:], identity=ident[:])
                        nc.scalar.copy(x2T[:, g * P:(g + 1) * P], pt[:])
                        bt = xtp.tile([P, 256], FP32, tag="bt")
                        for d in (0, 1):
                            pb = ps1p.tile([P, P], FP32, tag="pb")
                            nc.tensor.matmul(out=pb[:], lhsT=x2T[:, g * P:(g + 1) * P],
                                             rhs=wbot_sb[(1, d)][:], start=True, stop=True)
                            nc.scalar.copy(bt[:, d * F:(d + 1) * F], pb[:])
                        nc.sync.dma_start(out=T1own[g * P:g * P + rows, :],
                                          in_=bt[:rows, :])
                    else:
                        x3 = ltp.tile([P, F], FP32, tag="x2", name="x3")
                        nc.scalar.activation(x3[:], xn[:], AF.Relu)
                        pt = psp.tile([P, P], FP32, tag="ptr")
                        nc.tensor.transpose(out=pt[:], in_=x3[:], identity=ident[:])
                        x3T = xtp.tile([P, P], FP32, tag="x3T")
                        nc.scalar.copy(x3T[:], pt[:])
                        ph = ps1p.tile([P, 8], FP32, tag="pz")
                        nc.tensor.matmul(out=ph[:], lhsT=x3T[:], rhs=headw_sb[:],
                                         start=True, stop=True)
                        ot = ltp.tile([P, 8], FP32, tag="ot")
                        nc.vector.tensor_add(ot[:], ph[:], headb_sb[:])
                        nc.sync.dma_start(out=out_t[g * P:g * P + rows, :],
                                          in_=ot[:rows, :])

                if l == 0:
                    nc.gpsimd.collective_compute(
                        "AllGather", OP.bypass,
                        replica_groups=[list(range(NCORES))],
                        ins=[T1own[:]], outs=[T1[:]])

    nc.finalize()
    return nc


# ----------------------------------------------------------------- kernel()

def _host_prep(inputs):
    x = np.asarray(inputs["x"], np.float32)
    n_nodes, f = x.shape
    assert f == F and n_nodes % NCORES == 0
    nown = n_nodes // NCORES
    nownp = ((nown + P - 1) // P) * P
    ng = nownp // P
    lo_end = min(WIN, n_nodes)
    hi_start = max(n_nodes - WIN, 0)

    avg_in = float(np.asarray(inputs["avg_in"]))
    avg_out = float(np.asarray(inputs["avg_out"]))
    eis = {0: np.asarray(inputs["edge_index_in"], np.int64),
           1: np.asarray(inputs["edge_index_out"], np.int64)}

    core_asg, pos_asg, row = _assign_nodes(eis, n_nodes, nown, hi_start, lo_end)

    prep = {}
    KLd = {}
    KHd = {}
    empty_flag = {}
    for d, avg in ((0, avg_in), (1, avg_out)):
        cores, KL, KH, ef = _prep_direction(
            eis[d], core_asg, pos_asg, row, n_nodes, nown, nownp, ng,
            hi_start, lo_end, avg)
        prep[d] = cores
        KLd[d] = KL
        KHd[d] = KH
        empty_flag[d] = ef

    w = _pack_weights(inputs)

    # node order (natural id) per table row
    inv = np.empty(n_nodes, np.int64)
    inv[row] = np.arange(n_nodes)

    # layer-0 tables
    xperm = x[inv]                                     # [N,128] in row order
    T0 = np.concatenate([xperm @ w["wbot"][0, 0], xperm @ w["wbot"][0, 1]],
                        axis=1).astype(np.float32)     # [N, 256]
    A0 = np.zeros((NCORES, 2, nownp, 192), np.float32)
    xTown = np.zeros((NCORES, P, nownp), np.float32)
    for c in range(NCORES):
        xc = xperm[c * nown:(c + 1) * nown]            # [nown, 128] pos order
        for d in (0, 1):
            A0[c, d, :nown] = xc @ w["acatw"][0, d] + w["acatb"][0, d][0][None, :]
        xTown[c, :, :nown] = xc.T

    meta = dict(n_nodes=n_nodes, nown=nown, nownp=nownp, ng=ng,
                hi_start=hi_start, lo_end=lo_end, KL=KLd, KH=KHd,
                empty_flag=empty_flag)
    return meta, prep, w, T0, A0, xTown, core_asg, pos_asg


def _in_maps(meta, prep, w, T0, A0, xTown):
    maps = []
    for c in range(NCORES):
        m = dict(T0=T0, A0=A0[c], xTown=xTown[c],
                 acatw=w["acatw"], acatb=w["acatb"], ppw=w["ppw"],
                 linw=w["linw"], linb=w["linb"], wbot=w["wbot"],
                 combw1=w["combw1"], combw2=w["combw2"], combb=w["combb"],
                 headw=w["headw"], headb=w["headb"],
                 scal=np.stack([prep[0][c]["scal"], prep[1][c]["scal"]]))
        for d in (0, 1):
            m[f"idx_lo{d}"] = prep[d][c]["idx_lo"]
            m[f"idx_hi{d}"] = prep[d][c]["idx_hi"]
        maps.append(m)
    return maps


def _install_ntff_hook():
    import sys
    import types
    try:
        from antenv.axon_hooks import get_axon_ntff_profile_hook  # noqa: F401
        return
    except ImportError:
        pass
    try:
        mod = types.ModuleType("antenv.axon_hooks")
        hook = {"h": None}
        mod.set_axon_ntff_profile_hook = lambda h: hook.__setitem__("h", h)
        mod.get_axon_ntff_profile_hook = lambda: hook["h"]
        sys.modules["antenv.axon_hooks"] = mod
        import antenv
        antenv.axon_hooks = mod
        from trn_agent_boot.trn_boot import _ntff_profile_via_ctypes
        mod.set_axon_ntff_profile_hook(
            _ntff_profile_via_ctypes("/opt/axon/libaxon_pjrt.so"))
    except Exception:
        pass


def kernel(**inputs):
    meta, prep, w, T0, A0, xTown, core_asg, pos_asg = _host_prep(inputs)
    nc = build_program(meta)
    maps = _in_maps(meta, prep, w, T0, A0, xTown)

    trace = bool(int(os.environ.get("PNA_TRACE", "0")))
    if trace:
        _install_ntff_hook()
    res = run_bass_kernel_spmd(nc, maps, core_ids=list(range(NCORES)),
                               trace=trace)
    if trace and res.exec_time_ns is not None:
        print(f"HW exec time: {res.exec_time_ns} ns")
    n_nodes = meta["n_nodes"]
    nown = meta["nown"]
    out = np.empty((n_nodes, 8), np.float32)
    for c in range(NCORES):
        o = res.results[c]["out"]
        sel = core_asg == c
        out[sel] = o[pos_asg[sel]]
    return out.astype(np.float32)


def kernel_sim(**inputs):
    """MultiCoreSim path for small-input debugging."""
    from concourse.bass_interp import MultiCoreSim
    meta, prep, w, T0, A0, xTown, core_asg, pos_asg = _host_prep(inputs)
    nc = build_program(meta)
    maps = _in_maps(meta, prep, w, T0, A0, xTown)
    sim = MultiCoreSim(nc, num_cores=NCORES, trace=False,
                       require_finite=False, require_nnan=False)
    for c in range(NCORES):
        cs = sim.cores[c]
        for k, vv in maps[c].items():
            cs.tensor(k)[:] = vv
    sim.simulate(check_with_hw=False)
    n_nodes = meta["n_nodes"]
    out = np.empty((n_nodes, 8), np.float32)
    for c in range(NCORES):
        o = np.array(sim.cores[c].tensor("out"))
        sel = core_asg == c
        out[sel] = o[pos_asg[sel]]
    return out.astype(np.float32)
